# revision 28
# baseline (speedup 1.0000x reference)
"""Trainium2 Bass kernel for pre-LN multi-head attention (B=2, S=2048, H=1024, 16 heads).

Sharding: 8 cores = 2 batches x 4 query-blocks of 512 rows (no collectives).
All matmuls run in fp8e4m3 with DoubleRow perf mode (2 contraction tiles per
pass). LayerNorm is folded as ynT8 = fp8(x * rstd) plus a rank-1 correction
ride-along: contraction tile 8 of ynT8 holds ms = -mu*rstd*SM in partition 0,
and row 1024 of each (prescaled) weight matrix holds colsum(W8)/SM, so the
-mu part of LN is applied inside the projection matmuls. Weight matrices are
prescaled by WS=512 on the host (fp8 range), un-scaled at PSUM evacuation.
Softmax: exp on the Act engine over [128,1024] PSUM score regions (fp8 out),
denominator via an appended ones column on V, divided at context evacuation.
"""

import sys
import numpy as np
from contextlib import ExitStack

sys.path.insert(0, "/opt/trn_rl_repo")

import ml_dtypes  # noqa: E402
import concourse.bass as bass  # noqa: E402
import concourse.bacc as bacc  # noqa: E402
import concourse.tile as tile  # noqa: E402
from concourse import mybir  # noqa: E402

B, S, H = 2, 2048, 1024
HEADS, HD = 16, 64
NCORES = 8
SQ = 512          # query rows per core
HT = H // 128     # 8 hidden tiles
PAIRS = HEADS // 2
KCH = S // 128    # 16 key chunks of 128
WS = 512.0        # weight prescale (power of two, exact)
SM = 64.0         # correction-row scale split
F32 = mybir.dt.float32
F32R = mybir.dt.float32r
F8 = mybir.dt.float8e4
AF = mybir.ActivationFunctionType
OP = mybir.AluOpType
DRM = mybir.MatmulPerfMode.DoubleRow
E4 = ml_dtypes.float8_e4m3


def _r(ap):
    return ap.bitcast(F32R)


def slot0(ap):
    """[p, n] AP -> [p, 2, n] AP with stride-0 slot dim (reads data twice)."""
    return bass.AP(tensor=ap.tensor, offset=ap.offset,
                   ap=[list(ap.ap[0])] + [[0, 2]] + [list(d) for d in ap.ap[1:]])


def build_nc():
    nc = bacc.Bacc(num_swdge_queues=2)
    xT = nc.dram_tensor("xT", [H, S], F32R, kind="ExternalInput")
    xrb = nc.dram_tensor("xrb", [SQ, H], F32, kind="ExternalInput")
    wq8 = nc.dram_tensor("wq8", [1280, H], F8, kind="ExternalInput")
    wk8 = nc.dram_tensor("wk8", [1280, H], F8, kind="ExternalInput")
    wv8 = nc.dram_tensor("wv8", [1280, H], F8, kind="ExternalInput")
    wo8 = nc.dram_tensor("wo8", [H, H], F8, kind="ExternalInput")
    bq = nc.dram_tensor("bq", [H], F32, kind="ExternalInput")
    bk = nc.dram_tensor("bk", [H], F32, kind="ExternalInput")
    bv = nc.dram_tensor("bv", [H], F32, kind="ExternalInput")
    out = nc.dram_tensor("out", [SQ, H], F32, kind="ExternalOutput")

    xT_t = xT[:, :].rearrange("(t p) s -> p t s", p=128)        # [128, 8, 2048]
    wq8_t = wq8[:, :].rearrange("(t p) d -> p t d", p=128)      # [128, 10, 1024]
    wk8_t = wk8[:, :].rearrange("(t p) d -> p t d", p=128)
    wv8_t = wv8[:, :].rearrange("(t p) d -> p t d", p=128)
    wo8_t = wo8[:, :].rearrange("(t p) d -> p t d", p=128)      # [128, 8, 1024]
    xrb_t = xrb[:, :].rearrange("(t p) d -> t p d", p=128)

    def colvec(v):  # [H] dram -> [128, HT] sbuf col layout
        return v[:].rearrange("(t p) -> p t", p=128)

    inv_h = 1.0 / H

    with tile.TileContext(nc) as tc, ExitStack() as ctx:
        persist = ctx.enter_context(tc.tile_pool(name="persist", bufs=1))
        stream = ctx.enter_context(tc.tile_pool(name="stream", bufs=1))
        psum = ctx.enter_context(tc.tile_pool(name="psum", bufs=2, space="PSUM"))

        # ---- persistent sbuf ----
        ynT8 = persist.tile([128, 10, S], F8)       # fp8 x*rstd; tile8=ms row; tile9=0
        rstd_bc = persist.tile([128, S], F32)
        qt8 = persist.tile([128, PAIRS, SQ], F8)    # Q^T (pair-tiled)
        v8 = persist.tile([128, KCH, HEADS * 65], F8)   # V rows + ones col per head
        ctxT8 = persist.tile([128, HT, SQ], F8)
        kt8 = [persist.tile([128, 2, S], F8, name=f"kt8_{i}") for i in range(2)]
        wq8s = persist.tile([128, 10, H], F8)
        wk8s = persist.tile([128, 10, H], F8)
        wv8s = persist.tile([128, 10, H], F8)
        wo8s = persist.tile([128, HT, H], F8)
        bqcol = persist.tile([128, HT], F32)
        bkcol = persist.tile([128, HT], F32)
        bv_row = persist.tile([1, H], F32)
        bv_bc = persist.tile([128, H], F32)
        ones128 = persist.tile([128, 1], F32R)
        ones_f32 = persist.tile([128, 1], F32)
        eps_t = persist.tile([1, 1], F32)
        dummy = persist.tile([1, 1], F32)

        # ---- setup DMAs + memsets ----
        # x arrives via column-quarter groups: h-even tiles on the SP HWDGE
        # queue (engine busy for the transfer, but SP is otherwise idle),
        # h-odd via gpsimd SWDGE (cheap desc-gen, async transfer).
        nc.sync.dma_start(out=bqcol, in_=colvec(bq))
        nc.sync.dma_start(out=bkcol, in_=colvec(bk))
        nc.sync.dma_start(out=bv_row, in_=bv[:].rearrange("(o d) -> o d", o=1))
        xq = {}
        xos = []
        for q in range(4):
            sl = slice(q * 512, (q + 1) * 512)
            xe = stream.tile([128, 4, 512], F32R, tag="xqe", bufs=4, name="xqe")
            xo = stream.tile([128, 4, 512], F32R, tag="xqo", bufs=4, name="xqo")
            nc.sync.dma_start(out=xe, in_=xT_t[:, 0::2, sl])
            xos.append((xo, xT_t[:, 1::2, sl]))
            for h in range(HT):
                xq[(q, h)] = (xe if h % 2 == 0 else xo)[:, h // 2, :]
        # interleave weight transfers with the x h-odd groups on the SWDGE
        # queue so both arrive as their consumers come up
        nc.gpsimd.dma_start(out=wq8s, in_=wq8_t)
        nc.gpsimd.dma_start(out=xos[0][0], in_=xos[0][1])
        nc.gpsimd.dma_start(out=wk8s, in_=wk8_t)
        nc.gpsimd.dma_start(out=xos[1][0], in_=xos[1][1])
        nc.gpsimd.dma_start(out=wv8s, in_=wv8_t)
        nc.gpsimd.dma_start(out=xos[2][0], in_=xos[2][1])
        nc.gpsimd.dma_start(out=xos[3][0], in_=xos[3][1])
        nc.gpsimd.dma_start(out=wo8s, in_=wo8_t)
        nc.gpsimd.partition_broadcast(bv_bc, bv_row)
        nc.vector.memset(ones_f32, 1.0)
        nc.vector.tensor_copy(out=ones128, in_=ones_f32)
        nc.vector.memset(eps_t, 1e-5)
        # pull the sqrt/square table load early
        nc.scalar.activation(out=dummy, in_=eps_t, func=AF.Sqrt)
        v8_j = v8.rearrange("p k (j c) -> p k j c", c=65)
        # correction-row slots: tile 8 = ms row (part 0 written per quarter),
        # tile 9 = DoubleRow zero pad. DVE is idle at start.
        nc.vector.memset(ynT8[:, 8, :], 0.0)
        nc.vector.memset(ynT8[:, 9, :], 0.0)
        nc.vector.memset(kt8[0][:, 1, :], 0.0)  # DR slot-1 zeros (stay zero)

        # ---- phase 0: stats + prep, pipelined by column quarter ----

        def quarter(q, spool):
            sl = slice(q * 512, (q + 1) * 512)
            sacc = spool.tile([1, 512], F32, tag="sacc", name="sacc")
            qacc = spool.tile([1, 512], F32, tag="qacc", name="qacc")
            for h in range(HT):
                t_ = xq[(q, h)]
                xsq = stream.tile([128, 512], F32R, tag="xsq", bufs=2, name="xsq")
                # split squares between Act (idle pre-softmax) and gpsimd
                if h < 5:
                    nc.scalar.activation(out=xsq, in_=t_, func=AF.Square)
                else:
                    nc.gpsimd.tensor_mul(xsq, t_, t_)
                nc.tensor.matmul(sacc, ones128, t_,
                                 start=(h == 0), stop=(h == HT - 1))
                nc.tensor.matmul(qacc, ones128, xsq,
                                 start=(h == 0), stop=(h == HT - 1))
            tmp = stream.tile([1, 512], F32, tag="tmp", bufs=1, name="tmp")
            var = stream.tile([1, 512], F32, tag="var", bufs=1, name="var")
            rstd = stream.tile([1, 512], F32, tag="rstd", bufs=2, name="rstd")
            srow = stream.tile([1, 512], F32, tag="srow", bufs=1, name="srow")
            nc.vector.tensor_copy(out=srow, in_=sacc)
            nc.vector.scalar_tensor_tensor(out=tmp, in0=sacc, scalar=-inv_h * inv_h,
                                           in1=srow, op0=OP.mult, op1=OP.mult)
            nc.vector.scalar_tensor_tensor(out=var, in0=qacc, scalar=inv_h,
                                           in1=tmp, op0=OP.mult, op1=OP.add)
            nc.scalar.activation(out=var, in_=var, func=AF.Sqrt, bias=eps_t[:])
            nc.vector.reciprocal(out=rstd, in_=var)
            # ms row (fp8): -mu * rstd * SM  -> ynT8 tile 8, partition 0
            nc.vector.scalar_tensor_tensor(out=ynT8[0:1, 8, sl], in0=sacc,
                                           scalar=-inv_h * SM, in1=rstd,
                                           op0=OP.mult, op1=OP.mult)
            nc.gpsimd.partition_broadcast(rstd_bc[:, sl], rstd)
            # prep: ynT8 = fp8(x * rstd); split DVE/gpsimd per tile
            for h in range(HT):
                peng = nc.vector if (q == 0 or h < 4) else nc.gpsimd
                peng.tensor_mul(ynT8[:, h, sl], xq[(q, h)], rstd_bc[:, sl])

        def proj_group(w8s, t, movsl, dst_kind):
            """5 DoubleRow steps; movsl = column slice of ynT8 (as moving for
            q/k) or of the w (as moving for v/o).  dst_kind picks operand roles."""
            acc = psum.tile([128, 512], F32, tag="acc", name="acc")
            if dst_kind == "qk":   # out [128 dims, 512 cols]; moving = ynT8
                for i in range(5):
                    nc.tensor.matmul(acc[:, 0:movsl.stop - movsl.start],
                                     w8s[:, 2 * i:2 * i + 2, t * 128:(t + 1) * 128],
                                     ynT8[:, 2 * i:2 * i + 2, movsl],
                                     start=(i == 0), stop=(i == 4), perf_mode=DRM)
            else:                  # "vo": out [128 keys/q, 512 dims]; moving = W
                pass
            return acc

        # --- emission schedule ---
        with tc.tile_pool(name="statps", bufs=2, space="PSUM") as spool:
            quarter(0, spool)
            quarter(1, spool)

            # deferred memsets (keep them off the phase-0 critical path)
            nc.gpsimd.memset(kt8[1][:, 1, :], 0.0)  # DR slot-1 zeros (stay zero)
            # ones columns of V (denominator trick): v8[:, kc, j*65+64] = 1
            nc.gpsimd.memset(v8_j[:, :, :, 64:65], 1.0)

            # Q projection (own 512 query columns)
            for t in range(PAIRS):
                acc = proj_group(wq8s, t, slice(0, 512), "qk")
                nc.vector.tensor_scalar(out=qt8[:, t, :], in0=acc,
                                        scalar1=1.0 / WS, scalar2=bqcol[:, t:t + 1],
                                        op0=OP.mult, op1=OP.add)

            quarter(2, spool)

            def v_group(kc, jh):
                acc = psum.tile([128, 512], F32, tag="acc", name="acc_v")
                ksl = slice(kc * 128, (kc + 1) * 128)
                dsl = slice(jh * 512, (jh + 1) * 512)
                for i in range(5):
                    nc.tensor.matmul(acc, ynT8[:, 2 * i:2 * i + 2, ksl],
                                     wv8s[:, 2 * i:2 * i + 2, dsl],
                                     start=(i == 0), stop=(i == 4), perf_mode=DRM)
                # evac: (psum/WS) + bv -> v8 (8 heads x 64 dims, stride 65)
                nc.vector.scalar_tensor_tensor(
                    out=v8_j[:, kc, 8 * jh:8 * jh + 8, 0:64], in0=acc,
                    scalar=1.0 / WS, in1=bv_bc[:, dsl], op0=OP.mult, op1=OP.add)

            def k_group(pair, q, kbuf):
                sl = slice(q * 512, (q + 1) * 512)
                acc = proj_group(wk8s, pair, sl, "qk")
                nc.vector.tensor_scalar(out=kt8[kbuf][:, 0, sl], in0=acc,
                                        scalar1=1.0 / WS,
                                        scalar2=bkcol[:, pair:pair + 1],
                                        op0=OP.mult, op1=OP.add)

            for kc in range(0, 4):
                v_group(kc, 0)
                v_group(kc, 1)
            k_group(0, 0, 0)
            k_group(0, 1, 0)

            quarter(3, spool)

            for kc in range(4, 8):
                v_group(kc, 0)
                v_group(kc, 1)
            k_group(0, 2, 0)
            for kc in range(8, 12):
                v_group(kc, 0)
                v_group(kc, 1)
            k_group(0, 3, 0)
            for kc in range(12, 16):
                v_group(kc, 0)
                v_group(kc, 1)

        # warm the Exp table before the storm
        nc.scalar.activation(out=dummy, in_=eps_t, func=AF.Exp)

        with tc.tile_pool(name="regpool", bufs=2, space="PSUM") as rpool:

            def head(j, pair, kbuf, kwork):
                po = 64 * (j % 2)
                cps = psum.tile([65, 512], F32, tag="ctx", name="cps")
                qmov = slot0(qt8[po:po + 64, pair, :])
                pend_ctx = []   # emit ctx-DR one region late so an in-order PE
                                # stall on cps WAR never blocks the next scores

                def scores_exp(reg):
                    kc0 = 2 * reg
                    region = rpool.tile([128, 1024], F32, tag="region", name="reg")
                    nc.tensor.matmul(
                        region[:, 0:512],
                        kt8[kbuf][po:po + 64, :, kc0 * 128:(kc0 + 1) * 128],
                        qmov, start=True, stop=True, perf_mode=DRM)
                    nc.tensor.matmul(
                        region[:, 512:1024],
                        kt8[kbuf][po:po + 64, :, (kc0 + 1) * 128:(kc0 + 2) * 128],
                        qmov, start=True, stop=True, perf_mode=DRM)
                    et = stream.tile([128, 2, 512], F8, tag="et", bufs=3, name="et")
                    nc.scalar.activation(out=et, in_=region, func=AF.Exp,
                                         scale=0.125)
                    pend_ctx.append((reg, et))

                def ctx_dr():
                    reg, et = pend_ctx.pop(0)
                    kc0 = 2 * reg
                    nc.tensor.matmul(cps, v8[:, kc0:kc0 + 2, j * 65:j * 65 + 65],
                                     et, start=(reg == 0), stop=(reg == 7),
                                     perf_mode=DRM)

                for reg in range(8):
                    scores_exp(reg)
                    if reg >= 1:
                        ctx_dr()
                    if reg in (2, 5) and kwork:
                        kwork.pop(0)()
                ctx_dr()
                # evac: ctxT8 = fp8(cps[0:64] * (1/den))
                recip = stream.tile([1, 512], F32, tag="recip", bufs=2, name="recip")
                nc.vector.reciprocal(out=recip, in_=cps[64:65, :])
                rbc = stream.tile([64, 512], F32, tag="rbc", bufs=2, name="rbc")
                nc.gpsimd.partition_broadcast(rbc, recip)
                nc.vector.tensor_mul(ctxT8[po:po + 64, pair, :], cps[0:64, :], rbc)

            xr_tiles = {}
            ostash = {}

            def oproj_partial(qc, jh):
                # partial output projection over ctx pairs 0-5 (ready after
                # pair 5); stash = partial/WS + residual in SBUF
                dsl = slice(jh * 512, (jh + 1) * 512)
                acc = psum.tile([128, 512], F32, tag="acc", name="acc_op")
                for i in range(3):
                    nc.tensor.matmul(acc, ctxT8[:, 2 * i:2 * i + 2,
                                                qc * 128:(qc + 1) * 128],
                                     wo8s[:, 2 * i:2 * i + 2, dsl],
                                     start=(i == 0), stop=(i == 2),
                                     perf_mode=DRM)
                g = 2 * qc + jh
                if g % 4 == 0:
                    ostash["cur"] = stream.tile([128, 4, 512], F32R, tag="xqe",
                                                bufs=4, name="ostash")
                st_ = ostash["cur"][:, g % 4, :].bitcast(F32)
                nc.vector.scalar_tensor_tensor(out=st_, in0=acc, scalar=1.0 / WS,
                                               in1=xr_tiles[(qc, jh)],
                                               op0=OP.mult, op1=OP.add)
                ostash[(qc, jh)] = st_

            owork = []
            for pair in range(PAIRS):
                kbuf = pair % 2
                nbuf = (pair + 1) % 2
                kw = []
                if pair < PAIRS - 1:
                    kw = [lambda q=q: k_group(pair + 1, q, nbuf) for q in range(4)]
                elif pair == PAIRS - 1:
                    kw = owork
                head(2 * pair, pair, kbuf, kw)
                head(2 * pair + 1, pair, kbuf, kw)
                if pair == 4:
                    # prefetch residual tiles, recycling the dead x staging
                    for half in range(2):
                        xr = stream.tile([128, 4, 512], F32R, tag="xqo", bufs=4,
                                         name="xr")
                        nc.sync.dma_start(
                            out=xr.rearrange("p (t j) d -> p t j d", j=2),
                            in_=_r(xrb[half * 256:(half + 1) * 256, :]).rearrange(
                                "(t p) (j d) -> p t j d", p=128, d=512))
                        for s in range(4):
                            qc, jh = half * 2 + s // 2, s % 2
                            xr_tiles[(qc, jh)] = xr[:, s, :]
                if pair == 5:
                    owork = [lambda qc=qc, jh=jh: oproj_partial(qc, jh)
                             for qc in range(4) for jh in range(2)]

            for w in owork:   # any partials not emitted during pair 7
                w()

            # ---- output projection tail: ctx pairs 6-7 + stash + store ----
            for qc in range(4):
                for jh in range(2):
                    dsl = slice(jh * 512, (jh + 1) * 512)
                    acc = psum.tile([128, 512], F32, tag="acc", name="acc_o")
                    nc.tensor.matmul(acc, ctxT8[:, 6:8, qc * 128:(qc + 1) * 128],
                                     wo8s[:, 6:8, dsl],
                                     start=True, stop=True, perf_mode=DRM)
                    osb = stream.tile([128, 512], F32, tag="osb", bufs=2, name="osb")
                    nc.vector.scalar_tensor_tensor(out=osb, in0=acc, scalar=1.0 / WS,
                                                   in1=ostash[(qc, jh)],
                                                   op0=OP.mult, op1=OP.add)
                    eng = nc.sync if (qc + jh) % 2 == 0 else nc.gpsimd
                    eng.dma_start(
                        out=out[qc * 128:(qc + 1) * 128, dsl], in_=osb)
    nc.finalize()
    return nc


_NC = None


def _get_nc():
    global _NC
    if _NC is None:
        _NC = build_nc()
    return _NC


def _q8(a):
    return np.asarray(a, np.float32).astype(E4)


def make_in_maps(inputs):
    x = np.asarray(inputs["x"], np.float32)
    g = np.asarray(inputs["ln_g"], np.float32)
    lnb = np.asarray(inputs["ln_b"], np.float32)
    wq = np.asarray(inputs["Wq"], np.float32)
    wk = np.asarray(inputs["Wk"], np.float32)
    wv = np.asarray(inputs["Wv"], np.float32)
    wo = np.asarray(inputs["Wo"], np.float32)

    def prep_w(w):
        """[H,H] torch-layout W -> [1280,H] fp8: rows 0-1023 = fp8(WS*(W*g).T),
        row 1024 = fp8(colsum/SM), rest zero."""
        w8 = _q8(WS * (w * g).T)
        full = np.zeros((1280, H), E4)
        full[0:H] = w8
        full[H] = _q8(w8.astype(np.float32).sum(0) / SM)
        return full

    shared = {
        "wq8": prep_w(wq),
        "wk8": prep_w(wk),
        "wv8": prep_w(wv),
        "wo8": _q8(WS * wo.T),
        "bq": np.asarray(inputs["bq"], np.float32) + wq @ lnb,
        "bk": np.asarray(inputs["bk"], np.float32) + wk @ lnb,
        "bv": np.asarray(inputs["bv"], np.float32) + wv @ lnb,
    }
    bo = np.asarray(inputs["bo"], np.float32)
    in_maps = []
    for c in range(NCORES):
        b, q0 = c // 4, (c % 4) * SQ
        xbT = x[b].T  # [H, S]
        m = dict(shared)
        # roll so this core's own 512 query columns come first (SPMD: one
        # program, per-core data); attention is invariant to a consistent
        # permutation of the key/value axis.
        m["xT"] = np.ascontiguousarray(np.roll(xbT, -q0, axis=1))
        m["xrb"] = x[b, q0:q0 + SQ, :] + bo
        in_maps.append(m)
    return in_maps


def kernel(**inputs):
    from concourse.bass_utils import run_bass_kernel_spmd
    nc = _get_nc()
    in_maps = make_in_maps(inputs)
    res = run_bass_kernel_spmd(nc, in_maps, list(range(NCORES)))
    x = np.asarray(inputs["x"], np.float32)
    out = np.empty_like(x)
    for c in range(NCORES):
        b, q0 = c // 4, (c % 4) * SQ
        out[b, q0:q0 + SQ, :] = res.results[c]["out"]
    return out


# revision 32
# speedup vs baseline: 1.0355x; 1.0355x over previous
"""Trainium2 Bass kernel for pre-LN multi-head attention (B=2, S=2048, H=1024, 16 heads).

Sharding: 8 cores = 2 batches x 4 query-blocks of 512 rows (no collectives).
All matmuls run in fp8e4m3 DoubleRow (2 contraction tiles per pass, 0.5
cycles/row). LayerNorm: x^T ships as bf16; ynT8 = fp8(x * rstd); the -mu
correction rides as a K=1 matmul using ms = fp8(-mu*rstd*SM) against host-
provided colsum rows fp8(colsum(W8)/SM). Weights are prescaled by WS=512 on
the host, un-scaled at PSUM evacuation. rstd = exp(-0.5*ln(var+eps)) so the
whole kernel uses a single activation table set (ln/exp/square). Softmax:
exp over [128,1024] PSUM score regions straight to fp8; denominator via an
appended ones column on V. Pair 0's attention is interleaved with the
remaining LayerNorm quarters so the Act-engine exp storm starts at ~10us.
"""

import sys
import numpy as np
from contextlib import ExitStack

sys.path.insert(0, "/opt/trn_rl_repo")

import ml_dtypes  # noqa: E402
import concourse.bass as bass  # noqa: E402
import concourse.bacc as bacc  # noqa: E402
import concourse.tile as tile  # noqa: E402
from concourse import mybir  # noqa: E402

B, S, H = 2, 2048, 1024
HEADS, HD = 16, 64
NCORES = 8
SQ = 512          # query rows per core
HT = H // 128     # 8 hidden tiles
PAIRS = HEADS // 2
KCH = S // 128    # 16 key chunks of 128
WS = 512.0        # weight prescale (power of two, exact)
SM = 64.0         # correction-row scale split
F32 = mybir.dt.float32
F32R = mybir.dt.float32r
BF16 = mybir.dt.bfloat16
F8 = mybir.dt.float8e4
AF = mybir.ActivationFunctionType
OP = mybir.AluOpType
DRM = mybir.MatmulPerfMode.DoubleRow
E4 = ml_dtypes.float8_e4m3
BF = ml_dtypes.bfloat16


def slot0(ap):
    """[p, n] AP -> [p, 2, n] AP with stride-0 slot dim (reads data twice)."""
    return bass.AP(tensor=ap.tensor, offset=ap.offset,
                   ap=[list(ap.ap[0])] + [[0, 2]] + [list(d) for d in ap.ap[1:]])


def build_nc():
    nc = bacc.Bacc()
    xT = nc.dram_tensor("xT", [H, S], BF16, kind="ExternalInput")
    xrb = nc.dram_tensor("xrb", [SQ, H], F32, kind="ExternalInput")
    wq8 = nc.dram_tensor("wq8", [8, 128, HT, 128], F8, kind="ExternalInput")
    wk8 = nc.dram_tensor("wk8", [8, 128, HT, 128], F8, kind="ExternalInput")
    wv8 = nc.dram_tensor("wv8", [2, 128, HT, 512], F8, kind="ExternalInput")
    wo8 = nc.dram_tensor("wo8", [128, HT, H], F8, kind="ExternalInput")
    cs8 = nc.dram_tensor("cs8", [3, H], F8, kind="ExternalInput")
    bq = nc.dram_tensor("bq", [H], F32, kind="ExternalInput")
    bk = nc.dram_tensor("bk", [H], F32, kind="ExternalInput")
    bv = nc.dram_tensor("bv", [H], F32, kind="ExternalInput")
    out = nc.dram_tensor("out", [SQ, H], F32, kind="ExternalOutput")

    xT_t = xT[:, :].rearrange("(t p) s -> p t s", p=128)        # [128, 8, 2048]

    def colvec(v):
        return v[:].rearrange("(t p) -> p t", p=128)

    inv_h = 1.0 / H

    with tile.TileContext(nc) as tc, ExitStack() as ctx:
        persist = ctx.enter_context(tc.tile_pool(name="persist", bufs=1))
        stream = ctx.enter_context(tc.tile_pool(name="stream", bufs=1))
        psum = ctx.enter_context(tc.tile_pool(name="psum", bufs=1, space="PSUM"))

        # ---- persistent sbuf ----
        ynT8 = persist.tile([128, HT, S], F8)
        ms8 = persist.tile([1, S], F8)              # -mu*rstd*SM correction row
        rstd_bc = persist.tile([128, S], F32)
        qt8 = persist.tile([128, PAIRS, SQ], F8)
        v8 = persist.tile([128, KCH, HEADS * 65], F8)
        ctxT8 = persist.tile([128, HT, SQ], F8)
        kt8 = [persist.tile([128, 2, S], F8, name=f"kt8_{i}") for i in range(2)]
        wq8s = persist.tile([128, 8, HT, 128], F8)
        wk8s = persist.tile([128, 8, HT, 128], F8)
        wv8s = persist.tile([128, 2, HT, 512], F8)
        wo8s = persist.tile([128, HT, H], F8)
        csq8s = persist.tile([1, H], F8)
        csk8s = persist.tile([1, H], F8)
        csv8s = persist.tile([1, H], F8)
        bqcol = persist.tile([128, HT], F32)
        bkcol = persist.tile([128, HT], F32)
        bv_row = persist.tile([1, H], F32)
        bv_bc = persist.tile([128, H], F32)
        ones_bf = persist.tile([128, 1], BF16)
        eps_t = persist.tile([1, 1], F32)
        dummy = persist.tile([1, 1], F32)

        # ---- small setup ----
        nc.sync.dma_start(out=bqcol, in_=colvec(bq))
        nc.sync.dma_start(out=bkcol, in_=colvec(bk))
        nc.sync.dma_start(out=bv_row, in_=bv[:].rearrange("(o d) -> o d", o=1))
        nc.sync.dma_start(out=csq8s, in_=cs8[0:1, :])
        nc.sync.dma_start(out=csk8s, in_=cs8[1:2, :])
        nc.sync.dma_start(out=csv8s, in_=cs8[2:3, :])
        nc.vector.memset(ones_bf, 1.0)
        nc.vector.memset(eps_t, 1e-5)
        # single activation-table load for the whole kernel (ln/exp/square)
        nc.scalar.activation(out=dummy, in_=eps_t, func=AF.Ln)
        nc.gpsimd.partition_broadcast(bv_bc, bv_row)
        nc.gpsimd.memset(kt8[0][:, 1, :], 0.0)   # DR slot-1 zeros (stay zero)
        nc.gpsimd.memset(kt8[1][:, 1, :], 0.0)
        v8_j = v8.rearrange("p k (j c) -> p k j c", c=65)
        nc.gpsimd.memset(v8_j[:, :, :, 64:65], 1.0)  # softmax-denominator ones

        # ---- SP DMA sequencing: transfers execute in emission order ----
        xq = [stream.tile([128, HT, 512], BF16, tag="xq", bufs=4, name="xq")
              for _ in range(4)]

        def dma_x(q):
            nc.sync.dma_start(out=xq[q], in_=xT_t[:, :, q * 512:(q + 1) * 512])

        def dma_wchunk(w8s, wdram, c):
            nc.sync.dma_start(out=w8s[:, c, :, :], in_=wdram[c, :, :, :])

        dma_x(0)
        dma_wchunk(wq8s, wq8, 0)
        dma_wchunk(wk8s, wk8, 0)
        nc.sync.dma_start(out=wv8s[:, 0, :, :], in_=wv8[0, :, :, :])

        # ---- phase 0 pieces ----
        def quarter(q, spool):
            sl = slice(q * 512, (q + 1) * 512)
            st = spool.tile([33, 512], F32, tag="stat", bufs=1, name="stat")
            sacc, qacc = st[0:1, :], st[32:33, :]
            for h in range(HT):
                xsq = stream.tile([128, 512], BF16, tag="xsq", bufs=2, name="xsq")
                nc.vector.tensor_mul(xsq, xq[q][:, h, :], xq[q][:, h, :])
                nc.tensor.matmul(sacc, ones_bf, xq[q][:, h, :],
                                 start=(h == 0), stop=(h == HT - 1),
                                 skip_group_check=True)
                nc.tensor.matmul(qacc, ones_bf, xsq,
                                 start=(h == 0), stop=(h == HT - 1),
                                 skip_group_check=True)
            tmp = stream.tile([1, 512], F32, tag="tmp", bufs=1, name="tmp")
            var = stream.tile([1, 512], F32, tag="var", bufs=1, name="var")
            rstd = stream.tile([1, 512], F32, tag="rstd", bufs=2, name="rstd")
            srow = stream.tile([1, 512], F32, tag="srow", bufs=1, name="srow")
            nc.vector.tensor_copy(out=srow, in_=sacc)
            nc.vector.scalar_tensor_tensor(out=tmp, in0=sacc, scalar=-inv_h * inv_h,
                                           in1=srow, op0=OP.mult, op1=OP.mult)
            nc.vector.scalar_tensor_tensor(out=var, in0=qacc, scalar=inv_h,
                                           in1=tmp, op0=OP.mult, op1=OP.add)
            # rstd = exp(-0.5 * ln(var + eps)): stays in the ln/exp table set
            nc.scalar.activation(out=var, in_=var, func=AF.Ln, bias=eps_t[:])
            nc.scalar.activation(out=rstd, in_=var, func=AF.Exp, scale=-0.5)
            nc.vector.scalar_tensor_tensor(out=ms8[0:1, sl], in0=sacc,
                                           scalar=-inv_h * SM, in1=rstd,
                                           op0=OP.mult, op1=OP.mult)
            nc.gpsimd.partition_broadcast(rstd_bc[:, sl], rstd)
            for h in range(HT):
                peng = nc.vector if h < 3 else nc.gpsimd
                peng.tensor_mul(ynT8[:, h, sl], xq[q][:, h, :], rstd_bc[:, sl])

        def q_group(t):
            acc = psum.tile([128, 512], F32, tag="acc", bufs=1, name="acc_q")
            for i in range(4):
                nc.tensor.matmul(acc, wq8s[:, t, 2 * i:2 * i + 2, :],
                                 ynT8[:, 2 * i:2 * i + 2, 0:512],
                                 start=(i == 0), stop=False, perf_mode=DRM)
            nc.tensor.matmul(acc, csq8s[0:1, t * 128:(t + 1) * 128],
                             ms8[0:1, 0:512], start=False, stop=True)
            nc.vector.tensor_scalar(out=qt8[:, t, :], in0=acc,
                                    scalar1=1.0 / WS, scalar2=bqcol[:, t:t + 1],
                                    op0=OP.mult, op1=OP.add)

        def k_group(pair, q, kbuf):
            sl = slice(q * 512, (q + 1) * 512)
            acc = psum.tile([128, 512], F32, tag="acc", bufs=1, name="acc_k")
            for i in range(4):
                nc.tensor.matmul(acc, wk8s[:, pair, 2 * i:2 * i + 2, :],
                                 ynT8[:, 2 * i:2 * i + 2, sl],
                                 start=(i == 0), stop=False, perf_mode=DRM)
            nc.tensor.matmul(acc, csk8s[0:1, pair * 128:(pair + 1) * 128],
                             ms8[0:1, sl], start=False, stop=True)
            nc.vector.tensor_scalar(out=kt8[kbuf][:, 0, sl], in0=acc,
                                    scalar1=1.0 / WS,
                                    scalar2=bkcol[:, pair:pair + 1],
                                    op0=OP.mult, op1=OP.add)

        def v_group(kc, jh):
            ksl = slice(kc * 128, (kc + 1) * 128)
            acc = psum.tile([128, 512], F32, tag="acc", bufs=1, name="acc_v")
            for i in range(4):
                nc.tensor.matmul(acc, ynT8[:, 2 * i:2 * i + 2, ksl],
                                 wv8s[:, jh, 2 * i:2 * i + 2, :],
                                 start=(i == 0), stop=False, perf_mode=DRM)
            nc.tensor.matmul(acc, ms8[0:1, ksl],
                             csv8s[0:1, jh * 512:(jh + 1) * 512],
                             start=False, stop=True)
            nc.vector.scalar_tensor_tensor(
                out=v8_j[:, kc, 8 * jh:8 * jh + 8, 0:64], in0=acc,
                scalar=1.0 / WS, in1=bv_bc[:, jh * 512:(jh + 1) * 512],
                op0=OP.mult, op1=OP.add)

        # ---- attention head machinery (supports interleaved emission) ----
        rpool = ctx.enter_context(tc.tile_pool(name="regpool", bufs=2,
                                               space="PSUM"))
        spool = ctx.enter_context(tc.tile_pool(name="statps", bufs=1,
                                               space="PSUM"))

        class Head:
            def __init__(self, j, pair, kbuf):
                self.j, self.pair, self.kbuf = j, pair, kbuf
                self.po = 64 * (j % 2)
                self.cps = psum.tile([65, 512], F32, tag="ctx", bufs=2,
                                     name="cps")
                self.qmov = slot0(qt8[self.po:self.po + 64, pair, :])
                self.pend = []

            def scores_exp(self, reg):
                kc0 = 2 * reg
                po = self.po
                region = rpool.tile([128, 1024], F32, tag="region", name="reg")
                nc.tensor.matmul(
                    region[:, 0:512],
                    kt8[self.kbuf][po:po + 64, :, kc0 * 128:(kc0 + 1) * 128],
                    self.qmov, start=True, stop=True, perf_mode=DRM)
                nc.tensor.matmul(
                    region[:, 512:1024],
                    kt8[self.kbuf][po:po + 64, :, (kc0 + 1) * 128:(kc0 + 2) * 128],
                    self.qmov, start=True, stop=True, perf_mode=DRM)
                et = stream.tile([128, 2, 512], F8, tag="et", bufs=4, name="et")
                nc.scalar.activation(out=et, in_=region, func=AF.Exp, scale=0.125)
                self.pend.append((reg, et))

            def ctx_dr(self):
                reg, et = self.pend.pop(0)
                nc.tensor.matmul(self.cps,
                                 v8[:, 2 * reg:2 * reg + 2,
                                    self.j * 65:self.j * 65 + 65],
                                 et, start=(reg == 0), stop=(reg == 7),
                                 perf_mode=DRM)

            def evac(self):
                while self.pend:
                    self.ctx_dr()
                recip = stream.tile([1, 512], F32, tag="recip", bufs=2,
                                    name="recip")
                nc.vector.reciprocal(out=recip, in_=self.cps[64:65, :])
                rbc = stream.tile([64, 512], F32, tag="rbc", bufs=2, name="rbc")
                nc.gpsimd.partition_broadcast(rbc, recip)
                nc.vector.tensor_mul(ctxT8[self.po:self.po + 64, self.pair, :],
                                     self.cps[0:64, :], rbc)

        # --- pair 0 interleaved with the LayerNorm quarters ---
        quarter(0, spool)
        dma_x(1)
        dma_wchunk(wq8s, wq8, 1)
        dma_wchunk(wk8s, wk8, 1)
        q_group(0)
        k_group(0, 0, 0)
        for kc in range(0, 4):
            v_group(kc, 0)
        h0 = Head(0, 0, 0)
        h1 = Head(1, 0, 0)

        def p0_regions(lo, hi):
            for r in range(lo, hi):
                h0.scores_exp(r)
                if r > lo:
                    h0.ctx_dr()
                h1.scores_exp(r)
                if r > lo:
                    h1.ctx_dr()
            h0.ctx_dr()
            h1.ctx_dr()

        p0_regions(0, 2)
        quarter(1, spool)
        dma_x(2)
        for c in (2, 3):
            dma_wchunk(wq8s, wq8, c)
            dma_wchunk(wk8s, wk8, c)
        k_group(0, 1, 0)
        for kc in range(4, 8):
            v_group(kc, 0)
        p0_regions(2, 4)
        k_group(1, 0, 1)
        quarter(2, spool)
        dma_x(3)
        for c in (4, 5):
            dma_wchunk(wq8s, wq8, c)
            dma_wchunk(wk8s, wk8, c)
        k_group(0, 2, 0)
        for kc in range(8, 12):
            v_group(kc, 0)
        p0_regions(4, 6)
        k_group(1, 1, 1)
        quarter(3, spool)
        for c in (6, 7):
            dma_wchunk(wq8s, wq8, c)
            dma_wchunk(wk8s, wk8, c)
        nc.sync.dma_start(out=wv8s[:, 1, :, :], in_=wv8[1, :, :, :])
        nc.sync.dma_start(out=wo8s, in_=wo8[:, :, :])
        k_group(0, 3, 0)
        for kc in range(12, 16):
            v_group(kc, 0)
        p0_regions(6, 8)
        k_group(1, 2, 1)
        k_group(1, 3, 1)
        h0.evac()
        h1.evac()
        q_group(1)

        # --- pairs 1-7 with spread side-work ---
        xr_tiles = {}
        ostash = {}

        def oproj_partial(qc, jh):
            dsl = slice(jh * 512, (jh + 1) * 512)
            acc = psum.tile([128, 512], F32, tag="acc", bufs=1, name="acc_op")
            for i in range(3):
                nc.tensor.matmul(acc, ctxT8[:, 2 * i:2 * i + 2,
                                            qc * 128:(qc + 1) * 128],
                                 wo8s[:, 2 * i:2 * i + 2, dsl],
                                 start=(i == 0), stop=(i == 2),
                                 perf_mode=DRM)
            g = 2 * qc + jh
            nc.vector.scalar_tensor_tensor(out=ostash[g], in0=acc,
                                           scalar=1.0 / WS, in1=xr_tiles[g],
                                           op0=OP.mult, op1=OP.add)

        def head_run(j, pair, kbuf, work):
            hd = Head(j, pair, kbuf)
            for reg in range(8):
                hd.scores_exp(reg)
                if reg >= 1:
                    hd.ctx_dr()
                if reg >= 1 and work:
                    work.pop(0)()
            hd.ctx_dr()
            hd.evac()

        vwork = [lambda kc=kc: v_group(kc, 1) for kc in range(KCH)]
        pwork = {t: [] for t in range(1, PAIRS)}
        for t in range(1, PAIRS - 1):
            pwork[t] += [lambda q=q, t=t: k_group(t + 1, q, (t + 1) % 2)
                         for q in range(4)]
            pwork[t].append(lambda t=t: q_group(t + 1))
        for t in (1, 2, 3):
            pwork[t] += vwork[(t - 1) * 6:(t - 1) * 6 + 6]

        owork = []
        for pair in range(1, PAIRS):
            work = owork if pair == PAIRS - 1 else pwork[pair]
            head_run(2 * pair, pair, pair % 2, work)
            head_run(2 * pair + 1, pair, pair % 2, work)
            for w in work:
                w()
            work.clear()
            if pair == 4:
                # prefetch residual tiles into recycled x-staging tiles
                for half in range(2):
                    xrt = stream.tile([128, HT, 512], BF16, tag="xq", bufs=4,
                                      name="xrt")
                    xrf = xrt.rearrange("p t d -> p (t d)").bitcast(F32)
                    xrf = xrf.rearrange("p (g d) -> p g d", d=512)
                    nc.sync.dma_start(
                        out=xrf.rearrange("p (t j) d -> p t j d", j=2),
                        in_=xrb[half * 256:(half + 1) * 256, :].rearrange(
                            "(t p) (j d) -> p t j d", p=128, d=512))
                    for s in range(4):
                        xr_tiles[half * 4 + s] = xrf[:, s, :]
            if pair == 5:
                for half in range(2):
                    ost = stream.tile([128, HT, 512], BF16, tag="xq", bufs=4,
                                      name="ost")
                    osf = ost.rearrange("p t d -> p (t d)").bitcast(F32)
                    osf = osf.rearrange("p (g d) -> p g d", d=512)
                    for s in range(4):
                        ostash[half * 4 + s] = osf[:, s, :]
                owork += [lambda qc=qc, jh=jh: oproj_partial(qc, jh)
                          for qc in range(4) for jh in range(2)]

        # ---- output projection tail: ctx pairs 6-7 + stash + store ----
        for qc in range(4):
            for jh in range(2):
                dsl = slice(jh * 512, (jh + 1) * 512)
                acc = psum.tile([128, 512], F32, tag="acc", bufs=1,
                                name="acc_o")
                nc.tensor.matmul(acc, ctxT8[:, 6:8, qc * 128:(qc + 1) * 128],
                                 wo8s[:, 6:8, dsl],
                                 start=True, stop=True, perf_mode=DRM)
                osb = stream.tile([128, 512], F32, tag="osb", bufs=2,
                                  name="osb")
                nc.vector.scalar_tensor_tensor(out=osb, in0=acc,
                                               scalar=1.0 / WS,
                                               in1=ostash[2 * qc + jh],
                                               op0=OP.mult, op1=OP.add)
                eng = nc.sync if (qc + jh) % 2 == 0 else nc.gpsimd
                eng.dma_start(
                    out=out[qc * 128:(qc + 1) * 128, dsl], in_=osb)
    nc.finalize()
    return nc


_NC = None


def _get_nc():
    global _NC
    if _NC is None:
        _NC = build_nc()
    return _NC


def _q8(a):
    return np.asarray(a, np.float32).astype(E4)


def make_in_maps(inputs):
    x = np.asarray(inputs["x"], np.float32)
    g = np.asarray(inputs["ln_g"], np.float32)
    lnb = np.asarray(inputs["ln_b"], np.float32)
    wq = np.asarray(inputs["Wq"], np.float32)
    wk = np.asarray(inputs["Wk"], np.float32)
    wv = np.asarray(inputs["Wv"], np.float32)
    wo = np.asarray(inputs["Wo"], np.float32)

    wq8 = _q8(WS * (wq * g).T)    # [hidden, outdim]
    wk8 = _q8(WS * (wk * g).T)
    wv8 = _q8(WS * (wv * g).T)
    wo8 = _q8(WS * wo.T)
    cs8 = np.stack([_q8(w.astype(np.float32).sum(0) / SM)
                    for w in (wq8, wk8, wv8)])

    shared = {
        # chunk-major layouts so each DMA lands contiguous >=1KB runs
        "wq8": np.ascontiguousarray(
            wq8.reshape(8, 128, 8, 128).transpose(2, 1, 0, 3)),
        "wk8": np.ascontiguousarray(
            wk8.reshape(8, 128, 8, 128).transpose(2, 1, 0, 3)),
        "wv8": np.ascontiguousarray(
            wv8.reshape(8, 128, 2, 512).transpose(2, 1, 0, 3)),
        "wo8": np.ascontiguousarray(wo8.reshape(8, 128, H).transpose(1, 0, 2)),
        "cs8": cs8,
        "bq": np.asarray(inputs["bq"], np.float32) + wq @ lnb,
        "bk": np.asarray(inputs["bk"], np.float32) + wk @ lnb,
        "bv": np.asarray(inputs["bv"], np.float32) + wv @ lnb,
    }
    bo = np.asarray(inputs["bo"], np.float32)
    in_maps = []
    for c in range(NCORES):
        b, q0 = c // 4, (c % 4) * SQ
        xbT = x[b].T  # [H, S]
        m = dict(shared)
        # roll so this core's own 512 query columns come first; attention is
        # invariant to a consistent permutation of the key/value axis.
        m["xT"] = np.ascontiguousarray(np.roll(xbT, -q0, axis=1)).astype(BF)
        m["xrb"] = x[b, q0:q0 + SQ, :] + bo
        in_maps.append(m)
    return in_maps


def kernel(**inputs):
    from concourse.bass_utils import run_bass_kernel_spmd
    nc = _get_nc()
    in_maps = make_in_maps(inputs)
    res = run_bass_kernel_spmd(nc, in_maps, list(range(NCORES)))
    x = np.asarray(inputs["x"], np.float32)
    out = np.empty_like(x)
    for c in range(NCORES):
        b, q0 = c // 4, (c % 4) * SQ
        out[b, q0:q0 + SQ, :] = res.results[c]["out"]
    return out


# revision 37
# speedup vs baseline: 1.0846x; 1.0474x over previous
"""Trainium2 Bass kernel for pre-LN multi-head attention (B=2, S=2048, H=1024, 16 heads).

Sharding: 8 cores = 2 batches x 4 query-blocks of 512 rows (no collectives).
All matmuls run in fp8e4m3 DoubleRow (2 contraction tiles per pass, 0.5
cycles/row). LayerNorm: x^T ships as bf16; ynT8 = fp8(x * rstd); the -mu
correction rides as a K=1 matmul using ms = fp8(-mu*rstd*SM) against host-
provided colsum rows fp8(colsum(W8)/SM). Weights are prescaled by WS=512 on
the host, un-scaled at PSUM evacuation. rstd = exp(-0.5*ln(var+eps)) so the
whole kernel uses a single activation table set (ln/exp/square). Softmax:
exp over [128,1024] PSUM score regions straight to fp8; denominator via an
appended ones column on V. Pair 0's attention is interleaved with the
remaining LayerNorm quarters so the Act-engine exp storm starts at ~10us.
"""

import sys
import numpy as np
from contextlib import ExitStack

sys.path.insert(0, "/opt/trn_rl_repo")

import ml_dtypes  # noqa: E402
import concourse.bass as bass  # noqa: E402
import concourse.bacc as bacc  # noqa: E402
import concourse.tile as tile  # noqa: E402
from concourse import mybir  # noqa: E402

B, S, H = 2, 2048, 1024
HEADS, HD = 16, 64
NCORES = 8
SQ = 512          # query rows per core
HT = H // 128     # 8 hidden tiles
PAIRS = HEADS // 2
KCH = S // 128    # 16 key chunks of 128
WS = 512.0        # weight prescale (power of two, exact)
SM = 64.0         # correction-row scale split
F32 = mybir.dt.float32
F32R = mybir.dt.float32r
BF16 = mybir.dt.bfloat16
F8 = mybir.dt.float8e4
AF = mybir.ActivationFunctionType
OP = mybir.AluOpType
DRM = mybir.MatmulPerfMode.DoubleRow
E4 = ml_dtypes.float8_e4m3
BF = ml_dtypes.bfloat16


def slot0(ap):
    """[p, n] AP -> [p, 2, n] AP with stride-0 slot dim (reads data twice)."""
    return bass.AP(tensor=ap.tensor, offset=ap.offset,
                   ap=[list(ap.ap[0])] + [[0, 2]] + [list(d) for d in ap.ap[1:]])


def build_nc():
    nc = bacc.Bacc()
    xT = nc.dram_tensor("xT", [H, S], BF16, kind="ExternalInput")
    xrb = nc.dram_tensor("xrb", [SQ, H], F32, kind="ExternalInput")
    wq8 = nc.dram_tensor("wq8", [8, 128, HT, 128], F8, kind="ExternalInput")
    wk8 = nc.dram_tensor("wk8", [8, 128, HT, 128], F8, kind="ExternalInput")
    wv8 = nc.dram_tensor("wv8", [2, 128, HT, 512], F8, kind="ExternalInput")
    wo8 = nc.dram_tensor("wo8", [128, HT, H], F8, kind="ExternalInput")
    cs8 = nc.dram_tensor("cs8", [3, H], F8, kind="ExternalInput")
    bq = nc.dram_tensor("bq", [H], F32, kind="ExternalInput")
    bk = nc.dram_tensor("bk", [H], F32, kind="ExternalInput")
    bv = nc.dram_tensor("bv", [H], F32, kind="ExternalInput")
    out = nc.dram_tensor("out", [SQ, H], F32, kind="ExternalOutput")

    xT_t = xT[:, :].rearrange("(t p) s -> p t s", p=128)        # [128, 8, 2048]

    def colvec(v):
        return v[:].rearrange("(t p) -> p t", p=128)

    inv_h = 1.0 / H

    with tile.TileContext(nc) as tc, ExitStack() as ctx:
        persist = ctx.enter_context(tc.tile_pool(name="persist", bufs=1))
        stream = ctx.enter_context(tc.tile_pool(name="stream", bufs=1))
        psum = ctx.enter_context(tc.tile_pool(name="psum", bufs=1, space="PSUM"))

        # ---- persistent sbuf ----
        ynT8 = persist.tile([128, HT, S], F8)
        ms8 = persist.tile([1, S], F8)              # -mu*rstd*SM correction row
        rstd_bc = persist.tile([128, S], F32)
        qt8 = persist.tile([128, PAIRS, SQ], F8)
        v8 = persist.tile([128, KCH, HEADS * 65], F8)
        ctxT8 = persist.tile([128, HT, SQ], F8)
        kt8 = [persist.tile([128, 2, S], F8, name=f"kt8_{i}") for i in range(2)]
        wq8s = persist.tile([128, 8, HT, 128], F8)
        wk8s = persist.tile([128, 8, HT, 128], F8)
        wv8s = persist.tile([128, 2, HT, 512], F8)
        wo8s = persist.tile([128, HT, H], F8)
        csq8s = persist.tile([1, H], F8)
        csk8s = persist.tile([1, H], F8)
        csv8s = persist.tile([1, H], F8)
        bqcol = persist.tile([128, HT], F32)
        bkcol = persist.tile([128, HT], F32)
        bv_row = persist.tile([1, H], F32)
        bv_bc = persist.tile([128, H], F32)
        ones_bf = persist.tile([128, 1], BF16)
        eps_t = persist.tile([1, 1], F32)
        dummy = persist.tile([1, 1], F32)

        # ---- small setup ----
        nc.sync.dma_start(out=bqcol, in_=colvec(bq))
        nc.sync.dma_start(out=bkcol, in_=colvec(bk))
        nc.sync.dma_start(out=bv_row, in_=bv[:].rearrange("(o d) -> o d", o=1))
        nc.sync.dma_start(out=csq8s, in_=cs8[0:1, :])
        nc.sync.dma_start(out=csk8s, in_=cs8[1:2, :])
        nc.sync.dma_start(out=csv8s, in_=cs8[2:3, :])
        nc.vector.memset(ones_bf, 1.0)
        nc.vector.memset(eps_t, 1e-5)
        # single activation-table load for the whole kernel (ln/exp/square)
        nc.scalar.activation(out=dummy, in_=eps_t, func=AF.Ln)
        nc.gpsimd.partition_broadcast(bv_bc, bv_row)
        nc.gpsimd.memset(kt8[0][:, 1, :], 0.0)   # DR slot-1 zeros (stay zero)
        nc.gpsimd.memset(kt8[1][:, 1, :], 0.0)
        v8_j = v8.rearrange("p k (j c) -> p k j c", c=65)
        nc.gpsimd.memset(v8_j[:, :, :, 64:65], 1.0)  # softmax-denominator ones

        # ---- SP DMA sequencing: transfers execute in emission order ----
        xq = [stream.tile([128, HT, 512], BF16, tag="xq", bufs=4, name="xq")
              for _ in range(4)]

        def dma_x(q):
            nc.sync.dma_start(out=xq[q], in_=xT_t[:, :, q * 512:(q + 1) * 512])

        def dma_wchunk(w8s, wdram, c):
            nc.sync.dma_start(out=w8s[:, c, :, :], in_=wdram[c, :, :, :])

        prim = stream.tile([128, 512], BF16, tag="xsq", bufs=4, name="prim")
        nc.vector.memset(prim, 0.0)
        pacc = psum.tile([1, 512], F32, tag="acc", bufs=1, name="pacc")
        for i in range(10):
            nc.tensor.matmul(pacc, ones_bf, prim, start=(i == 0),
                             stop=(i == 9), skip_group_check=True)
        dma_x(0)
        dma_wchunk(wq8s, wq8, 0)
        dma_wchunk(wk8s, wk8, 0)
        nc.sync.dma_start(out=wv8s[:, 0, :, :], in_=wv8[0, :, :, :])

        # ---- phase 0 pieces ----
        def quarter(q, spool):
            sl = slice(q * 512, (q + 1) * 512)
            st = spool.tile([33, 512], F32, tag="stat", bufs=1, name="stat")
            sacc, qacc = st[0:1, :], st[32:33, :]
            for h in range(HT):
                nc.tensor.matmul(sacc, ones_bf, xq[q][:, h, :],
                                 start=(h == 0), stop=(h == HT - 1),
                                 skip_group_check=True)
            xsqs = []
            for h in range(HT):
                xsq = stream.tile([128, 512], BF16, tag="xsq", bufs=4, name="xsq")
                eng = nc.vector if h % 2 == 0 else nc.gpsimd
                eng.tensor_mul(xsq, xq[q][:, h, :], xq[q][:, h, :])
                xsqs.append(xsq)
            for h in range(HT):
                nc.tensor.matmul(qacc, ones_bf, xsqs[h],
                                 start=(h == 0), stop=(h == HT - 1),
                                 skip_group_check=True)
            # evacuate stat rows to SBUF once; epilogue runs on gpsimd
            srow = stream.tile([1, 512], F32, tag="srow", bufs=1, name="srow")
            qrow = stream.tile([1, 512], F32, tag="qrow", bufs=1, name="qrow")
            var = stream.tile([1, 512], F32, tag="var", bufs=1, name="var")
            rstd = stream.tile([1, 512], F32, tag="rstd", bufs=2, name="rstd")
            nc.vector.tensor_copy(out=srow, in_=sacc)
            nc.vector.tensor_copy(out=qrow, in_=qacc)
            nc.vector.scalar_tensor_tensor(out=var, in0=srow,
                                           scalar=-inv_h * inv_h,
                                           in1=srow, op0=OP.mult, op1=OP.mult)
            nc.vector.scalar_tensor_tensor(out=var, in0=qrow, scalar=inv_h,
                                           in1=var, op0=OP.mult, op1=OP.add)
            # rstd = exp(-0.5 * ln(var + eps)): stays in the ln/exp table set
            nc.scalar.activation(out=var, in_=var, func=AF.Ln, bias=eps_t[:])
            nc.scalar.activation(out=rstd, in_=var, func=AF.Exp, scale=-0.5)
            nc.vector.scalar_tensor_tensor(out=ms8[0:1, sl], in0=srow,
                                           scalar=-inv_h * SM, in1=rstd,
                                           op0=OP.mult, op1=OP.mult)
            nc.gpsimd.partition_broadcast(rstd_bc[:, sl], rstd)
            for h in range(HT):
                peng = nc.vector if h < 2 else nc.gpsimd
                peng.tensor_mul(ynT8[:, h, sl], xq[q][:, h, :], rstd_bc[:, sl])

        def q_group(t):
            acc = psum.tile([128, 512], F32, tag="acc", bufs=1, name="acc_q")
            for i in range(4):
                nc.tensor.matmul(acc, wq8s[:, t, 2 * i:2 * i + 2, :],
                                 ynT8[:, 2 * i:2 * i + 2, 0:512],
                                 start=(i == 0), stop=False, perf_mode=DRM)
            nc.tensor.matmul(acc, csq8s[0:1, t * 128:(t + 1) * 128],
                             ms8[0:1, 0:512], start=False, stop=True)
            nc.vector.tensor_scalar(out=qt8[:, t, :], in0=acc,
                                    scalar1=1.0 / WS, scalar2=bqcol[:, t:t + 1],
                                    op0=OP.mult, op1=OP.add)

        def k_group(pair, q, kbuf):
            sl = slice(q * 512, (q + 1) * 512)
            acc = psum.tile([128, 512], F32, tag="acc", bufs=1, name="acc_k")
            for i in range(4):
                nc.tensor.matmul(acc, wk8s[:, pair, 2 * i:2 * i + 2, :],
                                 ynT8[:, 2 * i:2 * i + 2, sl],
                                 start=(i == 0), stop=False, perf_mode=DRM)
            nc.tensor.matmul(acc, csk8s[0:1, pair * 128:(pair + 1) * 128],
                             ms8[0:1, sl], start=False, stop=True)
            nc.vector.tensor_scalar(out=kt8[kbuf][:, 0, sl], in0=acc,
                                    scalar1=1.0 / WS,
                                    scalar2=bkcol[:, pair:pair + 1],
                                    op0=OP.mult, op1=OP.add)

        def v_group(kc, jh):
            ksl = slice(kc * 128, (kc + 1) * 128)
            acc = psum.tile([128, 512], F32, tag="acc", bufs=1, name="acc_v")
            for i in range(4):
                nc.tensor.matmul(acc, ynT8[:, 2 * i:2 * i + 2, ksl],
                                 wv8s[:, jh, 2 * i:2 * i + 2, :],
                                 start=(i == 0), stop=False, perf_mode=DRM)
            nc.tensor.matmul(acc, ms8[0:1, ksl],
                             csv8s[0:1, jh * 512:(jh + 1) * 512],
                             start=False, stop=True)
            nc.vector.scalar_tensor_tensor(
                out=v8_j[:, kc, 8 * jh:8 * jh + 8, 0:64], in0=acc,
                scalar=1.0 / WS, in1=bv_bc[:, jh * 512:(jh + 1) * 512],
                op0=OP.mult, op1=OP.add)

        # ---- attention head machinery (supports interleaved emission) ----
        rpool = ctx.enter_context(tc.tile_pool(name="regpool", bufs=2,
                                               space="PSUM"))
        spool = ctx.enter_context(tc.tile_pool(name="statps", bufs=1,
                                               space="PSUM"))

        class Head:
            def __init__(self, j, pair, kbuf):
                self.j, self.pair, self.kbuf = j, pair, kbuf
                self.po = 64 * (j % 2)
                self.cps = psum.tile([65, 512], F32, tag="ctx", bufs=2,
                                     name="cps")
                self.qmov = slot0(qt8[self.po:self.po + 64, pair, :])
                self.pend = []

            def scores_exp(self, reg):
                kc0 = 2 * reg
                po = self.po
                region = rpool.tile([128, 1024], F32, tag="region", name="reg")
                nc.tensor.matmul(
                    region[:, 0:512],
                    kt8[self.kbuf][po:po + 64, :, kc0 * 128:(kc0 + 1) * 128],
                    self.qmov, start=True, stop=True, perf_mode=DRM)
                nc.tensor.matmul(
                    region[:, 512:1024],
                    kt8[self.kbuf][po:po + 64, :, (kc0 + 1) * 128:(kc0 + 2) * 128],
                    self.qmov, start=True, stop=True, perf_mode=DRM)
                et = stream.tile([128, 2, 512], F8, tag="et", bufs=4, name="et")
                nc.scalar.activation(out=et, in_=region, func=AF.Exp, scale=0.125)
                self.pend.append((reg, et))

            def ctx_dr(self):
                reg, et = self.pend.pop(0)
                nc.tensor.matmul(self.cps,
                                 v8[:, 2 * reg:2 * reg + 2,
                                    self.j * 65:self.j * 65 + 65],
                                 et, start=(reg == 0), stop=(reg == 7),
                                 perf_mode=DRM)

            def evac(self):
                while self.pend:
                    self.ctx_dr()
                recip = stream.tile([1, 512], F32, tag="recip", bufs=2,
                                    name="recip")
                nc.vector.reciprocal(out=recip, in_=self.cps[64:65, :])
                rbc = stream.tile([64, 512], F32, tag="rbc", bufs=2, name="rbc")
                nc.gpsimd.partition_broadcast(rbc, recip)
                nc.vector.tensor_mul(ctxT8[self.po:self.po + 64, self.pair, :],
                                     self.cps[0:64, :], rbc)

        # --- pair 0 interleaved with the LayerNorm quarters ---
        quarter(0, spool)
        dma_x(1)
        dma_wchunk(wq8s, wq8, 1)
        dma_wchunk(wk8s, wk8, 1)
        q_group(0)
        k_group(0, 0, 0)
        for kc in range(0, 4):
            v_group(kc, 0)
        h0 = Head(0, 0, 0)
        h1 = Head(1, 0, 0)

        def p0_regions(lo, hi):
            for r in range(lo, hi):
                h0.scores_exp(r)
                if r > lo:
                    h0.ctx_dr()
                h1.scores_exp(r)
                if r > lo:
                    h1.ctx_dr()
            h0.ctx_dr()
            h1.ctx_dr()

        p0_regions(0, 2)
        quarter(1, spool)
        dma_x(2)
        for c in (2, 3):
            dma_wchunk(wq8s, wq8, c)
            dma_wchunk(wk8s, wk8, c)
        k_group(0, 1, 0)
        for kc in range(4, 8):
            v_group(kc, 0)
        p0_regions(2, 4)
        k_group(1, 0, 1)
        quarter(2, spool)
        dma_x(3)
        for c in (4, 5):
            dma_wchunk(wq8s, wq8, c)
            dma_wchunk(wk8s, wk8, c)
        k_group(0, 2, 0)
        for kc in range(8, 12):
            v_group(kc, 0)
        p0_regions(4, 6)
        k_group(1, 1, 1)
        quarter(3, spool)
        for c in (6, 7):
            dma_wchunk(wq8s, wq8, c)
            dma_wchunk(wk8s, wk8, c)
        nc.sync.dma_start(out=wv8s[:, 1, :, :], in_=wv8[1, :, :, :])
        nc.sync.dma_start(out=wo8s, in_=wo8[:, :, :])
        k_group(0, 3, 0)
        for kc in range(12, 16):
            v_group(kc, 0)
        p0_regions(6, 8)
        k_group(1, 2, 1)
        k_group(1, 3, 1)
        h0.evac()
        h1.evac()
        q_group(1)

        # --- pairs 1-7 with spread side-work ---
        xr_tiles = {}
        ostash = {}

        def oproj_partial(qc, jh):
            dsl = slice(jh * 512, (jh + 1) * 512)
            acc = psum.tile([128, 512], F32, tag="acc", bufs=1, name="acc_op")
            for i in range(3):
                nc.tensor.matmul(acc, ctxT8[:, 2 * i:2 * i + 2,
                                            qc * 128:(qc + 1) * 128],
                                 wo8s[:, 2 * i:2 * i + 2, dsl],
                                 start=(i == 0), stop=(i == 2),
                                 perf_mode=DRM)
            g = 2 * qc + jh
            nc.vector.scalar_tensor_tensor(out=ostash[g], in0=acc,
                                           scalar=1.0 / WS, in1=xr_tiles[g],
                                           op0=OP.mult, op1=OP.add)

        def head_run(j, pair, kbuf, work):
            hd = Head(j, pair, kbuf)
            for reg in range(8):
                hd.scores_exp(reg)
                if reg >= 1:
                    hd.ctx_dr()
                if reg >= 1 and work:
                    work.pop(0)()
            hd.ctx_dr()
            hd.evac()

        vwork = [lambda kc=kc: v_group(kc, 1) for kc in range(KCH)]
        pwork = {t: [] for t in range(1, PAIRS)}
        for t in range(1, PAIRS - 1):
            pwork[t] += [lambda q=q, t=t: k_group(t + 1, q, (t + 1) % 2)
                         for q in range(4)]
            pwork[t].append(lambda t=t: q_group(t + 1))
        for t in (1, 2, 3):
            pwork[t] += vwork[(t - 1) * 6:(t - 1) * 6 + 6]

        owork = []
        for pair in range(1, PAIRS):
            work = owork if pair == PAIRS - 1 else pwork[pair]
            head_run(2 * pair, pair, pair % 2, work)
            head_run(2 * pair + 1, pair, pair % 2, work)
            for w in work:
                w()
            work.clear()
            if pair == 4:
                # prefetch residual tiles into recycled x-staging tiles
                for half in range(2):
                    xrt = stream.tile([128, HT, 512], BF16, tag="xq", bufs=4,
                                      name="xrt")
                    xrf = xrt.rearrange("p t d -> p (t d)").bitcast(F32)
                    xrf = xrf.rearrange("p (g d) -> p g d", d=512)
                    nc.sync.dma_start(
                        out=xrf.rearrange("p (t j) d -> p t j d", j=2),
                        in_=xrb[half * 256:(half + 1) * 256, :].rearrange(
                            "(t p) (j d) -> p t j d", p=128, d=512))
                    for s in range(4):
                        xr_tiles[half * 4 + s] = xrf[:, s, :]
            if pair == 5:
                for half in range(2):
                    ost = stream.tile([128, HT, 512], BF16, tag="xq", bufs=4,
                                      name="ost")
                    osf = ost.rearrange("p t d -> p (t d)").bitcast(F32)
                    osf = osf.rearrange("p (g d) -> p g d", d=512)
                    for s in range(4):
                        ostash[half * 4 + s] = osf[:, s, :]
                owork += [lambda qc=qc, jh=jh: oproj_partial(qc, jh)
                          for qc in range(4) for jh in range(2)]

        # ---- output projection tail: ctx pairs 6-7 + stash + store ----
        for qc in range(4):
            for jh in range(2):
                dsl = slice(jh * 512, (jh + 1) * 512)
                acc = psum.tile([128, 512], F32, tag="acc", bufs=1,
                                name="acc_o")
                nc.tensor.matmul(acc, ctxT8[:, 6:8, qc * 128:(qc + 1) * 128],
                                 wo8s[:, 6:8, dsl],
                                 start=True, stop=True, perf_mode=DRM)
                osb = stream.tile([128, 512], F32, tag="osb", bufs=2,
                                  name="osb")
                nc.vector.scalar_tensor_tensor(out=osb, in0=acc,
                                               scalar=1.0 / WS,
                                               in1=ostash[2 * qc + jh],
                                               op0=OP.mult, op1=OP.add)
                eng = nc.sync if (qc + jh) % 2 == 0 else nc.gpsimd
                eng.dma_start(
                    out=out[qc * 128:(qc + 1) * 128, dsl], in_=osb)
    nc.finalize()
    return nc


_NC = None


def _get_nc():
    global _NC
    if _NC is None:
        _NC = build_nc()
    return _NC


def _q8(a):
    return np.asarray(a, np.float32).astype(E4)


def make_in_maps(inputs):
    x = np.asarray(inputs["x"], np.float32)
    g = np.asarray(inputs["ln_g"], np.float32)
    lnb = np.asarray(inputs["ln_b"], np.float32)
    wq = np.asarray(inputs["Wq"], np.float32)
    wk = np.asarray(inputs["Wk"], np.float32)
    wv = np.asarray(inputs["Wv"], np.float32)
    wo = np.asarray(inputs["Wo"], np.float32)

    wq8 = _q8(WS * (wq * g).T)    # [hidden, outdim]
    wk8 = _q8(WS * (wk * g).T)
    wv8 = _q8(WS * (wv * g).T)
    wo8 = _q8(WS * wo.T)
    cs8 = np.stack([_q8(w.astype(np.float32).sum(0) / SM)
                    for w in (wq8, wk8, wv8)])

    shared = {
        # chunk-major layouts so each DMA lands contiguous >=1KB runs
        "wq8": np.ascontiguousarray(
            wq8.reshape(8, 128, 8, 128).transpose(2, 1, 0, 3)),
        "wk8": np.ascontiguousarray(
            wk8.reshape(8, 128, 8, 128).transpose(2, 1, 0, 3)),
        "wv8": np.ascontiguousarray(
            wv8.reshape(8, 128, 2, 512).transpose(2, 1, 0, 3)),
        "wo8": np.ascontiguousarray(wo8.reshape(8, 128, H).transpose(1, 0, 2)),
        "cs8": cs8,
        "bq": np.asarray(inputs["bq"], np.float32) + wq @ lnb,
        "bk": np.asarray(inputs["bk"], np.float32) + wk @ lnb,
        "bv": np.asarray(inputs["bv"], np.float32) + wv @ lnb,
    }
    bo = np.asarray(inputs["bo"], np.float32)
    in_maps = []
    for c in range(NCORES):
        b, q0 = c // 4, (c % 4) * SQ
        xbT = x[b].T  # [H, S]
        m = dict(shared)
        # roll so this core's own 512 query columns come first; attention is
        # invariant to a consistent permutation of the key/value axis.
        m["xT"] = np.ascontiguousarray(np.roll(xbT, -q0, axis=1)).astype(BF)
        m["xrb"] = x[b, q0:q0 + SQ, :] + bo
        in_maps.append(m)
    return in_maps


def kernel(**inputs):
    from concourse.bass_utils import run_bass_kernel_spmd
    nc = _get_nc()
    in_maps = make_in_maps(inputs)
    res = run_bass_kernel_spmd(nc, in_maps, list(range(NCORES)))
    x = np.asarray(inputs["x"], np.float32)
    out = np.empty_like(x)
    for c in range(NCORES):
        b, q0 = c // 4, (c % 4) * SQ
        out[b, q0:q0 + SQ, :] = res.results[c]["out"]
    return out


# revision 38
# speedup vs baseline: 1.0964x; 1.0110x over previous
"""Trainium2 Bass kernel for pre-LN multi-head attention (B=2, S=2048, H=1024, 16 heads).

Sharding: 8 cores = 2 batches x 4 query-blocks of 512 rows (no collectives).
All matmuls run in fp8e4m3 DoubleRow (2 contraction tiles per pass, 0.5
cycles/row). LayerNorm: x^T ships as bf16; ynT8 = fp8(x * rstd); the -mu
correction rides as a K=1 matmul using ms = fp8(-mu*rstd*SM) against host-
provided colsum rows fp8(colsum(W8)/SM). Weights are prescaled by WS=512 on
the host, un-scaled at PSUM evacuation. rstd = exp(-0.5*ln(var+eps)) so the
whole kernel uses a single activation table set (ln/exp/square). Softmax:
exp over [128,1024] PSUM score regions straight to fp8; denominator via an
appended ones column on V. Pair 0's attention is interleaved with the
remaining LayerNorm quarters so the Act-engine exp storm starts at ~10us.
"""

import sys
import numpy as np
from contextlib import ExitStack

sys.path.insert(0, "/opt/trn_rl_repo")

import ml_dtypes  # noqa: E402
import concourse.bass as bass  # noqa: E402
import concourse.bacc as bacc  # noqa: E402
import concourse.tile as tile  # noqa: E402
from concourse import mybir  # noqa: E402

B, S, H = 2, 2048, 1024
HEADS, HD = 16, 64
NCORES = 8
SQ = 512          # query rows per core
HT = H // 128     # 8 hidden tiles
PAIRS = HEADS // 2
KCH = S // 128    # 16 key chunks of 128
WS = 512.0        # weight prescale (power of two, exact)
SM = 64.0         # correction-row scale split
F32 = mybir.dt.float32
F32R = mybir.dt.float32r
BF16 = mybir.dt.bfloat16
F8 = mybir.dt.float8e4
AF = mybir.ActivationFunctionType
OP = mybir.AluOpType
DRM = mybir.MatmulPerfMode.DoubleRow
E4 = ml_dtypes.float8_e4m3
BF = ml_dtypes.bfloat16


def slot0(ap):
    """[p, n] AP -> [p, 2, n] AP with stride-0 slot dim (reads data twice)."""
    return bass.AP(tensor=ap.tensor, offset=ap.offset,
                   ap=[list(ap.ap[0])] + [[0, 2]] + [list(d) for d in ap.ap[1:]])


def build_nc():
    nc = bacc.Bacc()
    xT = nc.dram_tensor("xT", [H, S], BF16, kind="ExternalInput")
    xrb = nc.dram_tensor("xrb", [SQ, H], F32, kind="ExternalInput")
    wq8 = nc.dram_tensor("wq8", [8, 128, HT, 128], F8, kind="ExternalInput")
    wk8 = nc.dram_tensor("wk8", [8, 128, HT, 128], F8, kind="ExternalInput")
    wv8 = nc.dram_tensor("wv8", [2, 128, HT, 512], F8, kind="ExternalInput")
    wo8 = nc.dram_tensor("wo8", [128, HT, H], F8, kind="ExternalInput")
    cs8 = nc.dram_tensor("cs8", [3, H], F8, kind="ExternalInput")
    bq = nc.dram_tensor("bq", [H], F32, kind="ExternalInput")
    bk = nc.dram_tensor("bk", [H], F32, kind="ExternalInput")
    bv = nc.dram_tensor("bv", [H], F32, kind="ExternalInput")
    out = nc.dram_tensor("out", [SQ, H], F32, kind="ExternalOutput")

    xT_t = xT[:, :].rearrange("(t p) s -> p t s", p=128)        # [128, 8, 2048]

    def colvec(v):
        return v[:].rearrange("(t p) -> p t", p=128)

    inv_h = 1.0 / H

    with tile.TileContext(nc) as tc, ExitStack() as ctx:
        persist = ctx.enter_context(tc.tile_pool(name="persist", bufs=1))
        stream = ctx.enter_context(tc.tile_pool(name="stream", bufs=1))
        psum = ctx.enter_context(tc.tile_pool(name="psum", bufs=1, space="PSUM"))

        # ---- persistent sbuf ----
        ynT8 = persist.tile([128, HT, S], F8)
        ms8 = persist.tile([1, S], F8)              # -mu*rstd*SM correction row
        rstd_bc = persist.tile([128, S], F32)
        qt8 = persist.tile([128, PAIRS, SQ], F8)
        v8 = persist.tile([128, KCH, HEADS * 65], F8)
        ctxT8 = persist.tile([128, HT, SQ], F8)
        kt8 = [persist.tile([128, 2, S], F8, name=f"kt8_{i}") for i in range(2)]
        wq8s = persist.tile([128, 8, HT, 128], F8)
        wk8s = persist.tile([128, 8, HT, 128], F8)
        wv8s = persist.tile([128, 2, HT, 512], F8)
        wo8s = persist.tile([128, HT, H], F8)
        csq8s = persist.tile([1, H], F8)
        csk8s = persist.tile([1, H], F8)
        csv8s = persist.tile([1, H], F8)
        bqcol = persist.tile([128, HT], F32)
        bkcol = persist.tile([128, HT], F32)
        bv_row = persist.tile([1, H], F32)
        bv_bc = persist.tile([128, H], F32)
        ones_bf = persist.tile([128, 1], BF16)
        eps_t = persist.tile([1, 1], F32)
        dummy = persist.tile([1, 1], F32)

        # ---- small setup (bulk x/weight transfers get SP queue priority;
        # small tensors ride the gpsimd SWDGE queue) ----
        nc.vector.memset(ones_bf, 1.0)
        nc.vector.memset(eps_t, 1e-5)
        # single activation-table load for the whole kernel (ln/exp/square)
        nc.scalar.activation(out=dummy, in_=eps_t, func=AF.Ln)
        nc.gpsimd.dma_start(out=bv_row, in_=bv[:].rearrange("(o d) -> o d", o=1))
        nc.gpsimd.dma_start(out=bqcol, in_=colvec(bq))
        nc.gpsimd.dma_start(out=bkcol, in_=colvec(bk))
        nc.gpsimd.dma_start(out=csq8s, in_=cs8[0:1, :])
        nc.gpsimd.dma_start(out=csk8s, in_=cs8[1:2, :])
        nc.gpsimd.dma_start(out=csv8s, in_=cs8[2:3, :])
        nc.gpsimd.partition_broadcast(bv_bc, bv_row)
        nc.gpsimd.memset(kt8[0][:, 1, :], 0.0)   # DR slot-1 zeros (stay zero)
        nc.gpsimd.memset(kt8[1][:, 1, :], 0.0)
        v8_j = v8.rearrange("p k (j c) -> p k j c", c=65)
        nc.gpsimd.memset(v8_j[:, :, :, 64:65], 1.0)  # softmax-denominator ones

        # ---- SP DMA sequencing: transfers execute in emission order ----
        xq = [stream.tile([128, HT, 512], BF16, tag="xq", bufs=4, name="xq")
              for _ in range(4)]

        def dma_x(q):
            nc.sync.dma_start(out=xq[q], in_=xT_t[:, :, q * 512:(q + 1) * 512])

        def dma_wchunk(w8s, wdram, c):
            nc.sync.dma_start(out=w8s[:, c, :, :], in_=wdram[c, :, :, :])

        prim = stream.tile([128, 512], BF16, tag="xsq", bufs=4, name="prim")
        nc.vector.memset(prim, 0.0)
        pacc = psum.tile([1, 512], F32, tag="acc", bufs=1, name="pacc")
        for i in range(10):
            nc.tensor.matmul(pacc, ones_bf, prim, start=(i == 0),
                             stop=(i == 9), skip_group_check=True)
        dma_x(0)
        dma_wchunk(wq8s, wq8, 0)
        dma_wchunk(wk8s, wk8, 0)
        nc.sync.dma_start(out=wv8s[:, 0, :, :], in_=wv8[0, :, :, :])

        # ---- phase 0 pieces ----
        def quarter(q, spool):
            sl = slice(q * 512, (q + 1) * 512)
            st = spool.tile([33, 512], F32, tag="stat", bufs=1, name="stat")
            sacc, qacc = st[0:1, :], st[32:33, :]
            for h in range(HT):
                nc.tensor.matmul(sacc, ones_bf, xq[q][:, h, :],
                                 start=(h == 0), stop=(h == HT - 1),
                                 skip_group_check=True)
            xsqs = []
            for h in range(HT):
                xsq = stream.tile([128, 512], BF16, tag="xsq", bufs=4, name="xsq")
                eng = nc.vector if h % 2 == 0 else nc.gpsimd
                eng.tensor_mul(xsq, xq[q][:, h, :], xq[q][:, h, :])
                xsqs.append(xsq)
            for h in range(HT):
                nc.tensor.matmul(qacc, ones_bf, xsqs[h],
                                 start=(h == 0), stop=(h == HT - 1),
                                 skip_group_check=True)
            # evacuate stat rows to SBUF once; epilogue runs on gpsimd
            m_ = stream.tile([1, 512], F32, tag="srow", bufs=1, name="m_")
            var = stream.tile([1, 512], F32, tag="var", bufs=1, name="var")
            rstd = stream.tile([1, 512], F32, tag="rstd", bufs=2, name="rstd")
            nc.vector.tensor_scalar_mul(m_, sacc, inv_h)       # mean
            nc.vector.scalar_tensor_tensor(out=var, in0=m_, scalar=-1.0,
                                           in1=m_, op0=OP.mult, op1=OP.mult)
            nc.vector.scalar_tensor_tensor(out=var, in0=qacc, scalar=inv_h,
                                           in1=var, op0=OP.mult, op1=OP.add)
            # rstd = exp(-0.5 * ln(var + eps)): stays in the ln/exp table set
            nc.scalar.activation(out=var, in_=var, func=AF.Ln, bias=eps_t[:])
            nc.scalar.activation(out=rstd, in_=var, func=AF.Exp, scale=-0.5)
            nc.vector.scalar_tensor_tensor(out=ms8[0:1, sl], in0=m_,
                                           scalar=-SM, in1=rstd,
                                           op0=OP.mult, op1=OP.mult)
            nc.gpsimd.partition_broadcast(rstd_bc[:, sl], rstd)
            for h in range(HT):
                peng = nc.vector if h < 1 else nc.gpsimd
                peng.tensor_mul(ynT8[:, h, sl], xq[q][:, h, :], rstd_bc[:, sl])

        def q_group(t):
            acc = psum.tile([128, 512], F32, tag="acc", bufs=1, name="acc_q")
            for i in range(4):
                nc.tensor.matmul(acc, wq8s[:, t, 2 * i:2 * i + 2, :],
                                 ynT8[:, 2 * i:2 * i + 2, 0:512],
                                 start=(i == 0), stop=False, perf_mode=DRM)
            nc.tensor.matmul(acc, csq8s[0:1, t * 128:(t + 1) * 128],
                             ms8[0:1, 0:512], start=False, stop=True)
            nc.vector.tensor_scalar(out=qt8[:, t, :], in0=acc,
                                    scalar1=1.0 / WS, scalar2=bqcol[:, t:t + 1],
                                    op0=OP.mult, op1=OP.add)

        def k_group(pair, q, kbuf):
            sl = slice(q * 512, (q + 1) * 512)
            acc = psum.tile([128, 512], F32, tag="acc", bufs=1, name="acc_k")
            for i in range(4):
                nc.tensor.matmul(acc, wk8s[:, pair, 2 * i:2 * i + 2, :],
                                 ynT8[:, 2 * i:2 * i + 2, sl],
                                 start=(i == 0), stop=False, perf_mode=DRM)
            nc.tensor.matmul(acc, csk8s[0:1, pair * 128:(pair + 1) * 128],
                             ms8[0:1, sl], start=False, stop=True)
            nc.vector.tensor_scalar(out=kt8[kbuf][:, 0, sl], in0=acc,
                                    scalar1=1.0 / WS,
                                    scalar2=bkcol[:, pair:pair + 1],
                                    op0=OP.mult, op1=OP.add)

        def v_group(kc, jh):
            ksl = slice(kc * 128, (kc + 1) * 128)
            acc = psum.tile([128, 512], F32, tag="acc", bufs=1, name="acc_v")
            for i in range(4):
                nc.tensor.matmul(acc, ynT8[:, 2 * i:2 * i + 2, ksl],
                                 wv8s[:, jh, 2 * i:2 * i + 2, :],
                                 start=(i == 0), stop=False, perf_mode=DRM)
            nc.tensor.matmul(acc, ms8[0:1, ksl],
                             csv8s[0:1, jh * 512:(jh + 1) * 512],
                             start=False, stop=True)
            nc.vector.scalar_tensor_tensor(
                out=v8_j[:, kc, 8 * jh:8 * jh + 8, 0:64], in0=acc,
                scalar=1.0 / WS, in1=bv_bc[:, jh * 512:(jh + 1) * 512],
                op0=OP.mult, op1=OP.add)

        # ---- attention head machinery (supports interleaved emission) ----
        rpool = ctx.enter_context(tc.tile_pool(name="regpool", bufs=2,
                                               space="PSUM"))
        spool = ctx.enter_context(tc.tile_pool(name="statps", bufs=1,
                                               space="PSUM"))

        class Head:
            def __init__(self, j, pair, kbuf):
                self.j, self.pair, self.kbuf = j, pair, kbuf
                self.po = 64 * (j % 2)
                self.cps = psum.tile([65, 512], F32, tag="ctx", bufs=2,
                                     name="cps")
                self.qmov = slot0(qt8[self.po:self.po + 64, pair, :])
                self.pend = []

            def scores_exp(self, reg):
                kc0 = 2 * reg
                po = self.po
                region = rpool.tile([128, 1024], F32, tag="region", name="reg")
                nc.tensor.matmul(
                    region[:, 0:512],
                    kt8[self.kbuf][po:po + 64, :, kc0 * 128:(kc0 + 1) * 128],
                    self.qmov, start=True, stop=True, perf_mode=DRM)
                nc.tensor.matmul(
                    region[:, 512:1024],
                    kt8[self.kbuf][po:po + 64, :, (kc0 + 1) * 128:(kc0 + 2) * 128],
                    self.qmov, start=True, stop=True, perf_mode=DRM)
                et = stream.tile([128, 2, 512], F8, tag="et", bufs=4, name="et")
                nc.scalar.activation(out=et, in_=region, func=AF.Exp, scale=0.125)
                self.pend.append((reg, et))

            def ctx_dr(self):
                reg, et = self.pend.pop(0)
                nc.tensor.matmul(self.cps,
                                 v8[:, 2 * reg:2 * reg + 2,
                                    self.j * 65:self.j * 65 + 65],
                                 et, start=(reg == 0), stop=(reg == 7),
                                 perf_mode=DRM)

            def evac(self):
                while self.pend:
                    self.ctx_dr()
                recip = stream.tile([1, 512], F32, tag="recip", bufs=2,
                                    name="recip")
                nc.vector.reciprocal(out=recip, in_=self.cps[64:65, :])
                rbc = stream.tile([64, 512], F32, tag="rbc", bufs=2, name="rbc")
                nc.gpsimd.partition_broadcast(rbc, recip)
                nc.vector.tensor_mul(ctxT8[self.po:self.po + 64, self.pair, :],
                                     self.cps[0:64, :], rbc)

        # --- pair 0 interleaved with the LayerNorm quarters ---
        quarter(0, spool)
        dma_x(1)
        dma_wchunk(wq8s, wq8, 1)
        dma_wchunk(wk8s, wk8, 1)
        q_group(0)
        k_group(0, 0, 0)
        for kc in range(0, 4):
            v_group(kc, 0)
        h0 = Head(0, 0, 0)
        h1 = Head(1, 0, 0)

        def p0_regions(lo, hi):
            for r in range(lo, hi):
                h0.scores_exp(r)
                if r > lo:
                    h0.ctx_dr()
                h1.scores_exp(r)
                if r > lo:
                    h1.ctx_dr()
            h0.ctx_dr()
            h1.ctx_dr()

        p0_regions(0, 2)
        quarter(1, spool)
        dma_x(2)
        for c in (2, 3):
            dma_wchunk(wq8s, wq8, c)
            dma_wchunk(wk8s, wk8, c)
        k_group(0, 1, 0)
        for kc in range(4, 8):
            v_group(kc, 0)
        p0_regions(2, 4)
        k_group(1, 0, 1)
        quarter(2, spool)
        dma_x(3)
        for c in (4, 5):
            dma_wchunk(wq8s, wq8, c)
            dma_wchunk(wk8s, wk8, c)
        k_group(0, 2, 0)
        for kc in range(8, 12):
            v_group(kc, 0)
        p0_regions(4, 6)
        k_group(1, 1, 1)
        quarter(3, spool)
        for c in (6, 7):
            dma_wchunk(wq8s, wq8, c)
            dma_wchunk(wk8s, wk8, c)
        nc.sync.dma_start(out=wv8s[:, 1, :, :], in_=wv8[1, :, :, :])
        nc.sync.dma_start(out=wo8s, in_=wo8[:, :, :])
        k_group(0, 3, 0)
        for kc in range(12, 16):
            v_group(kc, 0)
        p0_regions(6, 8)
        k_group(1, 2, 1)
        k_group(1, 3, 1)
        h0.evac()
        h1.evac()
        q_group(1)

        # --- pairs 1-7 with spread side-work ---
        xr_tiles = {}
        ostash = {}

        def oproj_partial(qc, jh):
            dsl = slice(jh * 512, (jh + 1) * 512)
            acc = psum.tile([128, 512], F32, tag="acc", bufs=1, name="acc_op")
            for i in range(3):
                nc.tensor.matmul(acc, ctxT8[:, 2 * i:2 * i + 2,
                                            qc * 128:(qc + 1) * 128],
                                 wo8s[:, 2 * i:2 * i + 2, dsl],
                                 start=(i == 0), stop=(i == 2),
                                 perf_mode=DRM)
            g = 2 * qc + jh
            nc.vector.scalar_tensor_tensor(out=ostash[g], in0=acc,
                                           scalar=1.0 / WS, in1=xr_tiles[g],
                                           op0=OP.mult, op1=OP.add)

        def head_run(j, pair, kbuf, work):
            hd = Head(j, pair, kbuf)
            for reg in range(8):
                hd.scores_exp(reg)
                if reg >= 1:
                    hd.ctx_dr()
                if reg >= 1 and work:
                    work.pop(0)()
            hd.ctx_dr()
            hd.evac()

        vwork = [lambda kc=kc: v_group(kc, 1) for kc in range(KCH)]
        pwork = {t: [] for t in range(1, PAIRS)}
        for t in range(1, PAIRS - 1):
            pwork[t] += [lambda q=q, t=t: k_group(t + 1, q, (t + 1) % 2)
                         for q in range(4)]
            pwork[t].append(lambda t=t: q_group(t + 1))
        for t in (1, 2, 3):
            pwork[t] += vwork[(t - 1) * 6:(t - 1) * 6 + 6]

        owork = []
        for pair in range(1, PAIRS):
            work = owork if pair == PAIRS - 1 else pwork[pair]
            head_run(2 * pair, pair, pair % 2, work)
            head_run(2 * pair + 1, pair, pair % 2, work)
            for w in work:
                w()
            work.clear()
            if pair == 4:
                # prefetch residual tiles into recycled x-staging tiles
                for half in range(2):
                    xrt = stream.tile([128, HT, 512], BF16, tag="xq", bufs=4,
                                      name="xrt")
                    xrf = xrt.rearrange("p t d -> p (t d)").bitcast(F32)
                    xrf = xrf.rearrange("p (g d) -> p g d", d=512)
                    nc.sync.dma_start(
                        out=xrf.rearrange("p (t j) d -> p t j d", j=2),
                        in_=xrb[half * 256:(half + 1) * 256, :].rearrange(
                            "(t p) (j d) -> p t j d", p=128, d=512))
                    for s in range(4):
                        xr_tiles[half * 4 + s] = xrf[:, s, :]
            if pair == 5:
                for half in range(2):
                    ost = stream.tile([128, HT, 512], BF16, tag="xq", bufs=4,
                                      name="ost")
                    osf = ost.rearrange("p t d -> p (t d)").bitcast(F32)
                    osf = osf.rearrange("p (g d) -> p g d", d=512)
                    for s in range(4):
                        ostash[half * 4 + s] = osf[:, s, :]
                owork += [lambda qc=qc, jh=jh: oproj_partial(qc, jh)
                          for qc in range(4) for jh in range(2)]

        # ---- output projection tail: ctx pairs 6-7 + stash + store ----
        for qc in range(4):
            for jh in range(2):
                dsl = slice(jh * 512, (jh + 1) * 512)
                acc = psum.tile([128, 512], F32, tag="acc", bufs=1,
                                name="acc_o")
                nc.tensor.matmul(acc, ctxT8[:, 6:8, qc * 128:(qc + 1) * 128],
                                 wo8s[:, 6:8, dsl],
                                 start=True, stop=True, perf_mode=DRM)
                osb = stream.tile([128, 512], F32, tag="osb", bufs=2,
                                  name="osb")
                nc.vector.scalar_tensor_tensor(out=osb, in0=acc,
                                               scalar=1.0 / WS,
                                               in1=ostash[2 * qc + jh],
                                               op0=OP.mult, op1=OP.add)
                eng = nc.sync if (qc + jh) % 2 == 0 else nc.gpsimd
                eng.dma_start(
                    out=out[qc * 128:(qc + 1) * 128, dsl], in_=osb)
    nc.finalize()
    return nc


_NC = None


def _get_nc():
    global _NC
    if _NC is None:
        _NC = build_nc()
    return _NC


def _q8(a):
    return np.asarray(a, np.float32).astype(E4)


def make_in_maps(inputs):
    x = np.asarray(inputs["x"], np.float32)
    g = np.asarray(inputs["ln_g"], np.float32)
    lnb = np.asarray(inputs["ln_b"], np.float32)
    wq = np.asarray(inputs["Wq"], np.float32)
    wk = np.asarray(inputs["Wk"], np.float32)
    wv = np.asarray(inputs["Wv"], np.float32)
    wo = np.asarray(inputs["Wo"], np.float32)

    wq8 = _q8(WS * (wq * g).T)    # [hidden, outdim]
    wk8 = _q8(WS * (wk * g).T)
    wv8 = _q8(WS * (wv * g).T)
    wo8 = _q8(WS * wo.T)
    cs8 = np.stack([_q8(w.astype(np.float32).sum(0) / SM)
                    for w in (wq8, wk8, wv8)])

    shared = {
        # chunk-major layouts so each DMA lands contiguous >=1KB runs
        "wq8": np.ascontiguousarray(
            wq8.reshape(8, 128, 8, 128).transpose(2, 1, 0, 3)),
        "wk8": np.ascontiguousarray(
            wk8.reshape(8, 128, 8, 128).transpose(2, 1, 0, 3)),
        "wv8": np.ascontiguousarray(
            wv8.reshape(8, 128, 2, 512).transpose(2, 1, 0, 3)),
        "wo8": np.ascontiguousarray(wo8.reshape(8, 128, H).transpose(1, 0, 2)),
        "cs8": cs8,
        "bq": np.asarray(inputs["bq"], np.float32) + wq @ lnb,
        "bk": np.asarray(inputs["bk"], np.float32) + wk @ lnb,
        "bv": np.asarray(inputs["bv"], np.float32) + wv @ lnb,
    }
    bo = np.asarray(inputs["bo"], np.float32)
    in_maps = []
    for c in range(NCORES):
        b, q0 = c // 4, (c % 4) * SQ
        xbT = x[b].T  # [H, S]
        m = dict(shared)
        # roll so this core's own 512 query columns come first; attention is
        # invariant to a consistent permutation of the key/value axis.
        m["xT"] = np.ascontiguousarray(np.roll(xbT, -q0, axis=1)).astype(BF)
        m["xrb"] = x[b, q0:q0 + SQ, :] + bo
        in_maps.append(m)
    return in_maps


def kernel(**inputs):
    from concourse.bass_utils import run_bass_kernel_spmd
    nc = _get_nc()
    in_maps = make_in_maps(inputs)
    res = run_bass_kernel_spmd(nc, in_maps, list(range(NCORES)))
    x = np.asarray(inputs["x"], np.float32)
    out = np.empty_like(x)
    for c in range(NCORES):
        b, q0 = c // 4, (c % 4) * SQ
        out[b, q0:q0 + SQ, :] = res.results[c]["out"]
    return out


# revision 39
# speedup vs baseline: 1.1269x; 1.0278x over previous
"""Trainium2 Bass kernel for pre-LN multi-head attention (B=2, S=2048, H=1024, 16 heads).

Sharding: 8 cores = 2 batches x 4 query-blocks of 512 rows (no collectives).
All matmuls run in fp8e4m3 DoubleRow (2 contraction tiles per pass, 0.5
cycles/row). LayerNorm: x^T ships as bf16; ynT8 = fp8(x * rstd); the -mu
correction rides as a K=1 matmul using ms = fp8(-mu*rstd*SM) against host-
provided colsum rows fp8(colsum(W8)/SM). Weights are prescaled by WS=512 on
the host, un-scaled at PSUM evacuation. rstd = exp(-0.5*ln(var+eps)) so the
whole kernel uses a single activation table set (ln/exp/square). Softmax:
exp over [128,1024] PSUM score regions straight to fp8; denominator via an
appended ones column on V. Pair 0's attention is interleaved with the
remaining LayerNorm quarters so the Act-engine exp storm starts at ~10us.
"""

import sys
import numpy as np
from contextlib import ExitStack

sys.path.insert(0, "/opt/trn_rl_repo")

import ml_dtypes  # noqa: E402
import concourse.bass as bass  # noqa: E402
import concourse.bacc as bacc  # noqa: E402
import concourse.tile as tile  # noqa: E402
from concourse import mybir  # noqa: E402

B, S, H = 2, 2048, 1024
HEADS, HD = 16, 64
NCORES = 8
SQ = 512          # query rows per core
HT = H // 128     # 8 hidden tiles
PAIRS = HEADS // 2
KCH = S // 128    # 16 key chunks of 128
WS = 512.0        # weight prescale (power of two, exact)
SM = 64.0         # correction-row scale split
F32 = mybir.dt.float32
F32R = mybir.dt.float32r
BF16 = mybir.dt.bfloat16
F8 = mybir.dt.float8e4
AF = mybir.ActivationFunctionType
OP = mybir.AluOpType
DRM = mybir.MatmulPerfMode.DoubleRow
E4 = ml_dtypes.float8_e4m3
BF = ml_dtypes.bfloat16


def slot0(ap):
    """[p, n] AP -> [p, 2, n] AP with stride-0 slot dim (reads data twice)."""
    return bass.AP(tensor=ap.tensor, offset=ap.offset,
                   ap=[list(ap.ap[0])] + [[0, 2]] + [list(d) for d in ap.ap[1:]])


def build_nc():
    nc = bacc.Bacc()
    xT = nc.dram_tensor("xT", [H, S], BF16, kind="ExternalInput")
    xrb = nc.dram_tensor("xrb", [SQ, H], F32, kind="ExternalInput")
    wq8 = nc.dram_tensor("wq8", [8, 128, HT, 128], F8, kind="ExternalInput")
    wk8 = nc.dram_tensor("wk8", [8, 128, HT, 128], F8, kind="ExternalInput")
    wv8 = nc.dram_tensor("wv8", [2, 128, HT, 512], F8, kind="ExternalInput")
    wo8 = nc.dram_tensor("wo8", [128, HT, H], F8, kind="ExternalInput")
    cs8 = nc.dram_tensor("cs8", [3, H], F8, kind="ExternalInput")
    bq = nc.dram_tensor("bq", [H], F32, kind="ExternalInput")
    bk = nc.dram_tensor("bk", [H], F32, kind="ExternalInput")
    bv = nc.dram_tensor("bv", [H], F32, kind="ExternalInput")
    out = nc.dram_tensor("out", [SQ, H], F32, kind="ExternalOutput")

    xT_t = xT[:, :].rearrange("(t p) s -> p t s", p=128)        # [128, 8, 2048]

    def colvec(v):
        return v[:].rearrange("(t p) -> p t", p=128)

    inv_h = 1.0 / H

    with tile.TileContext(nc) as tc, ExitStack() as ctx:
        persist = ctx.enter_context(tc.tile_pool(name="persist", bufs=1))
        stream = ctx.enter_context(tc.tile_pool(name="stream", bufs=1))
        psum = ctx.enter_context(tc.tile_pool(name="psum", bufs=1, space="PSUM"))

        # ---- persistent sbuf ----
        ynT8 = persist.tile([128, HT, S], F8)
        ms8 = persist.tile([1, S], F8)              # -mu*rstd*SM correction row
        rstd_bc = persist.tile([128, S], F32)
        qt8 = persist.tile([128, PAIRS, SQ], F8)
        v8 = persist.tile([128, KCH, HEADS * 65], F8)
        ctxT8 = persist.tile([128, HT, SQ], F8)
        kt8 = [persist.tile([128, 2, S], F8, name=f"kt8_{i}") for i in range(2)]
        wq8s = persist.tile([128, 8, HT, 128], F8)
        wk8s = persist.tile([128, 8, HT, 128], F8)
        wv8s = persist.tile([128, 2, HT, 512], F8)
        wo8s = persist.tile([128, HT, H], F8)
        csq8s = persist.tile([1, H], F8)
        csk8s = persist.tile([1, H], F8)
        csv8s = persist.tile([1, H], F8)
        bqcol = persist.tile([128, HT], F32)
        bkcol = persist.tile([128, HT], F32)
        bv_row = persist.tile([1, H], F32)
        bv_bc = persist.tile([128, H], F32)
        ones_bf = persist.tile([128, 1], BF16)
        eps_t = persist.tile([1, 1], F32)
        dummy = persist.tile([1, 1], F32)

        # ---- small setup (bulk x/weight transfers get SP queue priority;
        # small tensors ride the gpsimd SWDGE queue) ----
        nc.vector.memset(ones_bf, 1.0)
        nc.vector.memset(eps_t, 1e-5)
        # single activation-table load for the whole kernel (ln/exp/square)
        nc.scalar.activation(out=dummy, in_=eps_t, func=AF.Ln)
        nc.scalar.dma_start(out=bv_row, in_=bv[:].rearrange("(o d) -> o d", o=1))
        nc.scalar.dma_start(out=bqcol, in_=colvec(bq))
        nc.scalar.dma_start(out=bkcol, in_=colvec(bk))
        nc.scalar.dma_start(out=csq8s, in_=cs8[0:1, :])
        nc.scalar.dma_start(out=csk8s, in_=cs8[1:2, :])
        nc.scalar.dma_start(out=csv8s, in_=cs8[2:3, :])
        v8_j = v8.rearrange("p k (j c) -> p k j c", c=65)

        # ---- SP DMA sequencing: transfers execute in emission order ----
        xq = [stream.tile([128, HT, 512], BF16, tag="xq", bufs=4, name="xq")
              for _ in range(4)]

        def dma_x(q):
            nc.sync.dma_start(out=xq[q], in_=xT_t[:, :, q * 512:(q + 1) * 512])

        def dma_wchunk(w8s, wdram, c):
            nc.sync.dma_start(out=w8s[:, c, :, :], in_=wdram[c, :, :, :])

        prim = stream.tile([128, 512], BF16, tag="xsq", bufs=4, name="prim")
        nc.vector.memset(prim, 0.0)
        pacc = psum.tile([1, 512], F32, tag="acc", bufs=1, name="pacc")
        for i in range(10):
            nc.tensor.matmul(pacc, ones_bf, prim, start=(i == 0),
                             stop=(i == 9), skip_group_check=True)
        dma_x(0)
        dma_wchunk(wq8s, wq8, 0)
        dma_wchunk(wk8s, wk8, 0)
        nc.sync.dma_start(out=wv8s[:, 0, :, :], in_=wv8[0, :, :, :])

        # ---- phase 0 pieces ----
        def quarter(q, spool):
            sl = slice(q * 512, (q + 1) * 512)
            st = spool.tile([33, 512], F32, tag="stat", bufs=1, name="stat")
            sacc, qacc = st[0:1, :], st[32:33, :]
            for h in range(HT):
                nc.tensor.matmul(sacc, ones_bf, xq[q][:, h, :],
                                 start=(h == 0), stop=(h == HT - 1),
                                 skip_group_check=True)
            xsqs = []
            for h in range(HT):
                xsq = stream.tile([128, 512], BF16, tag="xsq", bufs=4, name="xsq")
                eng = nc.vector if h % 2 == 0 else nc.gpsimd
                eng.tensor_mul(xsq, xq[q][:, h, :], xq[q][:, h, :])
                xsqs.append(xsq)
            for h in range(HT):
                nc.tensor.matmul(qacc, ones_bf, xsqs[h],
                                 start=(h == 0), stop=(h == HT - 1),
                                 skip_group_check=True)
            # evacuate stat rows to SBUF once; epilogue runs on gpsimd
            m_ = stream.tile([1, 512], F32, tag="srow", bufs=1, name="m_")
            var = stream.tile([1, 512], F32, tag="var", bufs=1, name="var")
            rstd = stream.tile([1, 512], F32, tag="rstd", bufs=2, name="rstd")
            nc.vector.tensor_scalar_mul(m_, sacc, inv_h)       # mean
            nc.vector.scalar_tensor_tensor(out=var, in0=m_, scalar=-1.0,
                                           in1=m_, op0=OP.mult, op1=OP.mult)
            nc.vector.scalar_tensor_tensor(out=var, in0=qacc, scalar=inv_h,
                                           in1=var, op0=OP.mult, op1=OP.add)
            # rstd = exp(-0.5 * ln(var + eps)): stays in the ln/exp table set
            nc.scalar.activation(out=var, in_=var, func=AF.Ln, bias=eps_t[:])
            nc.scalar.activation(out=rstd, in_=var, func=AF.Exp, scale=-0.5)
            nc.vector.scalar_tensor_tensor(out=ms8[0:1, sl], in0=m_,
                                           scalar=-SM, in1=rstd,
                                           op0=OP.mult, op1=OP.mult)
            nc.gpsimd.partition_broadcast(rstd_bc[:, sl], rstd)
            for h in range(HT):
                peng = nc.vector if h < 1 else nc.gpsimd
                peng.tensor_mul(ynT8[:, h, sl], xq[q][:, h, :], rstd_bc[:, sl])

        def q_group(t):
            acc = psum.tile([128, 512], F32, tag="acc", bufs=1, name="acc_q")
            for i in range(4):
                nc.tensor.matmul(acc, wq8s[:, t, 2 * i:2 * i + 2, :],
                                 ynT8[:, 2 * i:2 * i + 2, 0:512],
                                 start=(i == 0), stop=False, perf_mode=DRM)
            nc.tensor.matmul(acc, csq8s[0:1, t * 128:(t + 1) * 128],
                             ms8[0:1, 0:512], start=False, stop=True)
            nc.vector.tensor_scalar(out=qt8[:, t, :], in0=acc,
                                    scalar1=1.0 / WS, scalar2=bqcol[:, t:t + 1],
                                    op0=OP.mult, op1=OP.add)

        def k_group(pair, q, kbuf):
            sl = slice(q * 512, (q + 1) * 512)
            acc = psum.tile([128, 512], F32, tag="acc", bufs=1, name="acc_k")
            for i in range(4):
                nc.tensor.matmul(acc, wk8s[:, pair, 2 * i:2 * i + 2, :],
                                 ynT8[:, 2 * i:2 * i + 2, sl],
                                 start=(i == 0), stop=False, perf_mode=DRM)
            nc.tensor.matmul(acc, csk8s[0:1, pair * 128:(pair + 1) * 128],
                             ms8[0:1, sl], start=False, stop=True)
            nc.vector.tensor_scalar(out=kt8[kbuf][:, 0, sl], in0=acc,
                                    scalar1=1.0 / WS,
                                    scalar2=bkcol[:, pair:pair + 1],
                                    op0=OP.mult, op1=OP.add)

        def v_group(kc, jh):
            ksl = slice(kc * 128, (kc + 1) * 128)
            acc = psum.tile([128, 512], F32, tag="acc", bufs=1, name="acc_v")
            for i in range(4):
                nc.tensor.matmul(acc, ynT8[:, 2 * i:2 * i + 2, ksl],
                                 wv8s[:, jh, 2 * i:2 * i + 2, :],
                                 start=(i == 0), stop=False, perf_mode=DRM)
            nc.tensor.matmul(acc, ms8[0:1, ksl],
                             csv8s[0:1, jh * 512:(jh + 1) * 512],
                             start=False, stop=True)
            nc.vector.scalar_tensor_tensor(
                out=v8_j[:, kc, 8 * jh:8 * jh + 8, 0:64], in0=acc,
                scalar=1.0 / WS, in1=bv_bc[:, jh * 512:(jh + 1) * 512],
                op0=OP.mult, op1=OP.add)

        # ---- attention head machinery (supports interleaved emission) ----
        rpool = ctx.enter_context(tc.tile_pool(name="regpool", bufs=2,
                                               space="PSUM"))
        spool = ctx.enter_context(tc.tile_pool(name="statps", bufs=1,
                                               space="PSUM"))

        class Head:
            def __init__(self, j, pair, kbuf):
                self.j, self.pair, self.kbuf = j, pair, kbuf
                self.po = 64 * (j % 2)
                self.cps = psum.tile([65, 512], F32, tag="ctx", bufs=2,
                                     name="cps")
                self.qmov = slot0(qt8[self.po:self.po + 64, pair, :])
                self.pend = []

            def scores_exp(self, reg):
                kc0 = 2 * reg
                po = self.po
                region = rpool.tile([128, 1024], F32, tag="region", name="reg")
                nc.tensor.matmul(
                    region[:, 0:512],
                    kt8[self.kbuf][po:po + 64, :, kc0 * 128:(kc0 + 1) * 128],
                    self.qmov, start=True, stop=True, perf_mode=DRM)
                nc.tensor.matmul(
                    region[:, 512:1024],
                    kt8[self.kbuf][po:po + 64, :, (kc0 + 1) * 128:(kc0 + 2) * 128],
                    self.qmov, start=True, stop=True, perf_mode=DRM)
                et = stream.tile([128, 2, 512], F8, tag="et", bufs=4, name="et")
                nc.scalar.activation(out=et, in_=region, func=AF.Exp, scale=0.125)
                self.pend.append((reg, et))

            def ctx_dr(self):
                reg, et = self.pend.pop(0)
                nc.tensor.matmul(self.cps,
                                 v8[:, 2 * reg:2 * reg + 2,
                                    self.j * 65:self.j * 65 + 65],
                                 et, start=(reg == 0), stop=(reg == 7),
                                 perf_mode=DRM)

            def evac(self):
                while self.pend:
                    self.ctx_dr()
                recip = stream.tile([1, 512], F32, tag="recip", bufs=2,
                                    name="recip")
                nc.vector.reciprocal(out=recip, in_=self.cps[64:65, :])
                rbc = stream.tile([64, 512], F32, tag="rbc", bufs=2, name="rbc")
                nc.gpsimd.partition_broadcast(rbc, recip)
                nc.vector.tensor_mul(ctxT8[self.po:self.po + 64, self.pair, :],
                                     self.cps[0:64, :], rbc)

        # --- pair 0 interleaved with the LayerNorm quarters ---
        quarter(0, spool)
        nc.gpsimd.partition_broadcast(bv_bc, bv_row)
        nc.gpsimd.memset(kt8[0][:, 1, :], 0.0)   # DR slot-1 zeros (stay zero)
        nc.gpsimd.memset(kt8[1][:, 1, :], 0.0)
        nc.gpsimd.memset(v8_j[:, :, :, 64:65], 1.0)  # softmax-denominator ones
        dma_x(1)
        dma_wchunk(wq8s, wq8, 1)
        dma_wchunk(wk8s, wk8, 1)
        q_group(0)
        k_group(0, 0, 0)
        for kc in range(0, 4):
            v_group(kc, 0)
        h0 = Head(0, 0, 0)
        h1 = Head(1, 0, 0)

        def p0_regions(lo, hi):
            for r in range(lo, hi):
                h0.scores_exp(r)
                if r > lo:
                    h0.ctx_dr()
                h1.scores_exp(r)
                if r > lo:
                    h1.ctx_dr()
            h0.ctx_dr()
            h1.ctx_dr()

        p0_regions(0, 2)
        quarter(1, spool)
        dma_x(2)
        for c in (2, 3):
            dma_wchunk(wq8s, wq8, c)
            dma_wchunk(wk8s, wk8, c)
        k_group(0, 1, 0)
        for kc in range(4, 8):
            v_group(kc, 0)
        p0_regions(2, 4)
        k_group(1, 0, 1)
        quarter(2, spool)
        dma_x(3)
        for c in (4, 5):
            dma_wchunk(wq8s, wq8, c)
            dma_wchunk(wk8s, wk8, c)
        k_group(0, 2, 0)
        for kc in range(8, 12):
            v_group(kc, 0)
        p0_regions(4, 6)
        k_group(1, 1, 1)
        quarter(3, spool)
        for c in (6, 7):
            dma_wchunk(wq8s, wq8, c)
            dma_wchunk(wk8s, wk8, c)
        nc.sync.dma_start(out=wv8s[:, 1, :, :], in_=wv8[1, :, :, :])
        nc.sync.dma_start(out=wo8s, in_=wo8[:, :, :])
        k_group(0, 3, 0)
        for kc in range(12, 16):
            v_group(kc, 0)
        p0_regions(6, 8)
        k_group(1, 2, 1)
        k_group(1, 3, 1)
        h0.evac()
        h1.evac()
        q_group(1)

        # --- pairs 1-7 with spread side-work ---
        xr_tiles = {}
        ostash = {}

        def oproj_partial(qc, jh):
            dsl = slice(jh * 512, (jh + 1) * 512)
            acc = psum.tile([128, 512], F32, tag="acc", bufs=1, name="acc_op")
            for i in range(3):
                nc.tensor.matmul(acc, ctxT8[:, 2 * i:2 * i + 2,
                                            qc * 128:(qc + 1) * 128],
                                 wo8s[:, 2 * i:2 * i + 2, dsl],
                                 start=(i == 0), stop=(i == 2),
                                 perf_mode=DRM)
            g = 2 * qc + jh
            nc.vector.scalar_tensor_tensor(out=ostash[g], in0=acc,
                                           scalar=1.0 / WS, in1=xr_tiles[g],
                                           op0=OP.mult, op1=OP.add)

        def head_run(j, pair, kbuf, work):
            hd = Head(j, pair, kbuf)
            for reg in range(8):
                hd.scores_exp(reg)
                if reg >= 1:
                    hd.ctx_dr()
                if reg >= 1 and work:
                    work.pop(0)()
            hd.ctx_dr()
            hd.evac()

        vwork = [lambda kc=kc: v_group(kc, 1) for kc in range(KCH)]
        pwork = {t: [] for t in range(1, PAIRS)}
        for t in range(1, PAIRS - 1):
            pwork[t] += [lambda q=q, t=t: k_group(t + 1, q, (t + 1) % 2)
                         for q in range(4)]
            pwork[t].append(lambda t=t: q_group(t + 1))
        for t in (1, 2, 3):
            pwork[t] += vwork[(t - 1) * 6:(t - 1) * 6 + 6]

        owork = []
        for pair in range(1, PAIRS):
            work = owork if pair == PAIRS - 1 else pwork[pair]
            head_run(2 * pair, pair, pair % 2, work)
            head_run(2 * pair + 1, pair, pair % 2, work)
            for w in work:
                w()
            work.clear()
            if pair == 4:
                # prefetch residual tiles into recycled x-staging tiles
                for half in range(2):
                    xrt = stream.tile([128, HT, 512], BF16, tag="xq", bufs=4,
                                      name="xrt")
                    xrf = xrt.rearrange("p t d -> p (t d)").bitcast(F32)
                    xrf = xrf.rearrange("p (g d) -> p g d", d=512)
                    nc.sync.dma_start(
                        out=xrf.rearrange("p (t j) d -> p t j d", j=2),
                        in_=xrb[half * 256:(half + 1) * 256, :].rearrange(
                            "(t p) (j d) -> p t j d", p=128, d=512))
                    for s in range(4):
                        xr_tiles[half * 4 + s] = xrf[:, s, :]
            if pair == 5:
                for half in range(2):
                    ost = stream.tile([128, HT, 512], BF16, tag="xq", bufs=4,
                                      name="ost")
                    osf = ost.rearrange("p t d -> p (t d)").bitcast(F32)
                    osf = osf.rearrange("p (g d) -> p g d", d=512)
                    for s in range(4):
                        ostash[half * 4 + s] = osf[:, s, :]
                owork += [lambda qc=qc, jh=jh: oproj_partial(qc, jh)
                          for qc in range(4) for jh in range(2)]

        # ---- output projection tail: ctx pairs 6-7 + stash + store ----
        for qc in range(4):
            for jh in range(2):
                dsl = slice(jh * 512, (jh + 1) * 512)
                acc = psum.tile([128, 512], F32, tag="acc", bufs=1,
                                name="acc_o")
                nc.tensor.matmul(acc, ctxT8[:, 6:8, qc * 128:(qc + 1) * 128],
                                 wo8s[:, 6:8, dsl],
                                 start=True, stop=True, perf_mode=DRM)
                osb = stream.tile([128, 512], F32, tag="osb", bufs=2,
                                  name="osb")
                nc.vector.scalar_tensor_tensor(out=osb, in0=acc,
                                               scalar=1.0 / WS,
                                               in1=ostash[2 * qc + jh],
                                               op0=OP.mult, op1=OP.add)
                eng = nc.sync if (qc + jh) % 2 == 0 else nc.gpsimd
                eng.dma_start(
                    out=out[qc * 128:(qc + 1) * 128, dsl], in_=osb)
    nc.finalize()
    return nc


_NC = None


def _get_nc():
    global _NC
    if _NC is None:
        _NC = build_nc()
    return _NC


def _q8(a):
    return np.asarray(a, np.float32).astype(E4)


def make_in_maps(inputs):
    x = np.asarray(inputs["x"], np.float32)
    g = np.asarray(inputs["ln_g"], np.float32)
    lnb = np.asarray(inputs["ln_b"], np.float32)
    wq = np.asarray(inputs["Wq"], np.float32)
    wk = np.asarray(inputs["Wk"], np.float32)
    wv = np.asarray(inputs["Wv"], np.float32)
    wo = np.asarray(inputs["Wo"], np.float32)

    wq8 = _q8(WS * (wq * g).T)    # [hidden, outdim]
    wk8 = _q8(WS * (wk * g).T)
    wv8 = _q8(WS * (wv * g).T)
    wo8 = _q8(WS * wo.T)
    cs8 = np.stack([_q8(w.astype(np.float32).sum(0) / SM)
                    for w in (wq8, wk8, wv8)])

    shared = {
        # chunk-major layouts so each DMA lands contiguous >=1KB runs
        "wq8": np.ascontiguousarray(
            wq8.reshape(8, 128, 8, 128).transpose(2, 1, 0, 3)),
        "wk8": np.ascontiguousarray(
            wk8.reshape(8, 128, 8, 128).transpose(2, 1, 0, 3)),
        "wv8": np.ascontiguousarray(
            wv8.reshape(8, 128, 2, 512).transpose(2, 1, 0, 3)),
        "wo8": np.ascontiguousarray(wo8.reshape(8, 128, H).transpose(1, 0, 2)),
        "cs8": cs8,
        "bq": np.asarray(inputs["bq"], np.float32) + wq @ lnb,
        "bk": np.asarray(inputs["bk"], np.float32) + wk @ lnb,
        "bv": np.asarray(inputs["bv"], np.float32) + wv @ lnb,
    }
    bo = np.asarray(inputs["bo"], np.float32)
    in_maps = []
    for c in range(NCORES):
        b, q0 = c // 4, (c % 4) * SQ
        xbT = x[b].T  # [H, S]
        m = dict(shared)
        # roll so this core's own 512 query columns come first; attention is
        # invariant to a consistent permutation of the key/value axis.
        m["xT"] = np.ascontiguousarray(np.roll(xbT, -q0, axis=1)).astype(BF)
        m["xrb"] = x[b, q0:q0 + SQ, :] + bo
        in_maps.append(m)
    return in_maps


def kernel(**inputs):
    from concourse.bass_utils import run_bass_kernel_spmd
    nc = _get_nc()
    in_maps = make_in_maps(inputs)
    res = run_bass_kernel_spmd(nc, in_maps, list(range(NCORES)))
    x = np.asarray(inputs["x"], np.float32)
    out = np.empty_like(x)
    for c in range(NCORES):
        b, q0 = c // 4, (c % 4) * SQ
        out[b, q0:q0 + SQ, :] = res.results[c]["out"]
    return out
